# revision 52
# baseline (speedup 1.0000x reference)
"""DGCNN forward on 8 Trainium2 NeuronCores (Bass/Tile), pure data parallel.

Each core processes one sample (N=1024 points, K=20 neighbors).

Algorithmic mapping per EdgeConv layer (weights BN-folded on host):
  y[:,n,j] = Wd@(x_nbr - x_ctr) + Wc@x_ctr   (1x1 conv on edge features)
           = Wd@x[:,idx[n,j]] + (Wc-Wd)@x[:,n]
  After folding the (eval-mode) BN scale s and bias into the weights, and
  because max over neighbors commutes with the monotone LeakyReLU:
  out[:,n] = lrelu( max_j u[:,idx[n,j]] + v[:,n] )
  with u = (s*Wd)@x + (s*bias + b)  and  v = (s*(Wc-Wd))@x.

  KNN row scores: top-20 of  s[n,m] = <x_n,x_m> - ||x_m||^2/2  (equivalent
  ordering to the reference's -||x_n-x_m||^2 per row).

Top-20 per row: 3 rounds of DVE max8 / max_index / match_replace.
Neighbor gather: gpsimd dma_gather of u^T rows from DRAM, split over the
4 SWDGE queues; reduce-max over the 20 gathered rows on the Pool engine.
"""

import os

import numpy as np

N = 1024
K = 20
NCORES = 8
EPS = 1e-5
SLOPE = 0.01
NEG = -3.0e38

# (C_in, O) per edge conv layer
EDGE_LAYERS = [(3, 64), (64, 64), (64, 128), (128, 256)]

_CACHE = {}
LAST_RESULTS = None


def _build():
    import concourse.bass as bass
    import concourse.mybir as mybir
    import concourse.tile as tile
    from concourse import bacc

    dt = mybir.dt
    f32 = dt.float32
    u16 = dt.uint16
    f16 = dt.float16
    i16 = dt.int16
    Alu = mybir.AluOpType
    Act = mybir.ActivationFunctionType
    AX = mybir.AxisListType
    f32r = dt.float32r

    nc = bacc.Bacc("TRN2", target_bir_lowering=False, debug=False,
                   num_swdge_queues=4)
    from concourse.masks import make_identity

    def mmr(out, lhsT, rhs, **kw):
        # fp32 matmul at full PE rate via the float32r replication mode
        nc.tensor.matmul(out, lhsT.bitcast(f32r), rhs.bitcast(f32r), **kw)

    # ---------------- DRAM I/O ----------------
    xin = nc.dram_tensor("xin", [3, N], f32, kind="ExternalInput")
    ATs, BTs, c0s = [], [], []
    for li, (C, O) in enumerate(EDGE_LAYERS):
        ATs.append(nc.dram_tensor(f"AT{li}", [C, O], f32, kind="ExternalInput"))
        BTs.append(nc.dram_tensor(f"BT{li}", [C, O], f32, kind="ExternalInput"))
        c0s.append(nc.dram_tensor(f"c0{li}", [1, O], f32, kind="ExternalInput"))
    # conv5 + MLP head run in fp16 (full PE rate); weights cast on host
    w5T = nc.dram_tensor("w5T", [512, 1024], f16, kind="ExternalInput")
    l1T = nc.dram_tensor("l1T", [1024, 512], f16, kind="ExternalInput")
    b6 = nc.dram_tensor("b6", [1, 512], f16, kind="ExternalInput")
    l2T = nc.dram_tensor("l2T", [512, 256], f16, kind="ExternalInput")
    c7 = nc.dram_tensor("c7", [1, 256], f16, kind="ExternalInput")
    l3T = nc.dram_tensor("l3T", [256, 40], f16, kind="ExternalInput")
    b8 = nc.dram_tensor("b8", [1, 40], f16, kind="ExternalInput")
    out_d = nc.dram_tensor("out", [40, 1], f32, kind="ExternalOutput")

    with tile.TileContext(nc) as tc, __import__("contextlib").ExitStack() as ctx:
        const = ctx.enter_context(tc.tile_pool(name="const", bufs=1))
        xpool = ctx.enter_context(tc.tile_pool(name="xpool", bufs=1))
        work = ctx.enter_context(tc.tile_pool(name="work", bufs=2))
        srow_p = ctx.enter_context(tc.tile_pool(name="srow", bufs=4))
        gth_p = ctx.enter_context(tc.tile_pool(name="gth", bufs=20))
        vt_p = ctx.enter_context(tc.tile_pool(name="vt", bufs=3))
        small = ctx.enter_context(tc.tile_pool(name="small", bufs=4))
        mm = ctx.enter_context(tc.tile_pool(name="mm", bufs=4, space="PSUM"))
        sm = ctx.enter_context(tc.tile_pool(name="sm", bufs=3, space="PSUM"))
        dram = ctx.enter_context(tc.tile_pool(name="dram", bufs=2, space="DRAM"))
        dram_s = ctx.enter_context(tc.tile_pool(name="dram_s", bufs=3, space="DRAM"))

        # persistent channel-major feature tiles; layers whose KNN input has
        # C<128 carry an extra all-ones channel row used to fold the -xx/2
        # rank-1 term into the single distance matmul (lhsT side).
        # x0 is padded to 33 rows: x in rows 0:3, zeros in 3:32 (so they
        # contribute nothing to the K=33 distance matmul), ones row at 32
        # (engine writes must start at a 32-aligned partition).
        x0 = xpool.tile([33, 1024], f32, tag="x0")
        x1 = xpool.tile([65, 1024], f32, tag="x1")
        x2 = xpool.tile([65, 1024], f32, tag="x2")
        x3 = xpool.tile([128, 1024], f32, tag="x3")
        x4a = xpool.tile([128, 1024], f32, tag="x4a")
        x4b = xpool.tile([128, 1024], f32, tag="x4b")
        gp = xpool.tile([128, 8], f32, tag="gp")
        nc.vector.memset(x0[:], 0.0)
        nc.vector.memset(x0[32:33, :], 1.0)
        nc.vector.memset(x1[64:65, :], 1.0)
        nc.vector.memset(x2[64:65, :], 1.0)

        # load x (host pre-transposed channel-major [3, 1024]; a device-side
        # transposing DMA would be 4B-granule descriptor hell) FIRST so L1
        # isn't blocked behind the ~3MB of weight loads below.
        nc.sync.dma_start(x0[0:3, :], xin.ap())

        # ------------- constants into SBUF -------------
        def load_const(name, dram_t, shape=None, dtype=None):
            t = const.tile(list(shape or dram_t.shape), dtype or f32, tag=name)
            nc.sync.dma_start(t[:], dram_t.ap())
            return t

        AT_sb = [load_const(f"AT{i}", ATs[i]) for i in range(4)]
        BT_sb = [load_const(f"BT{i}", BTs[i]) for i in range(4)]
        c0_sb = [load_const(f"c0{i}", c0s[i]) for i in range(4)]
        b6_sb = load_const("b6", b6, dtype=f16)
        c7_sb = load_const("c7", c7, dtype=f16)
        b8_sb = load_const("b8", b8, dtype=f16)

        # w5T: 5 K-chunks matching [x1(64), x2(64), x3(128), x4a(128), x4b(128)]
        # fp16: conv5 + head run at full PE rate
        w5_rows = [(0, 64), (64, 128), (128, 256), (256, 384), (384, 512)]
        w5_sb = []
        for i, (r0, r1) in enumerate(w5_rows):
            t = const.tile([r1 - r0, 1024], f16, tag=f"w5_{i}")
            nc.sync.dma_start(t[:], w5T.ap()[r0:r1, :])
            w5_sb.append(t)
        l1_sb = []
        for k in range(8):
            t = const.tile([128, 512], f16, tag=f"l1_{k}")
            nc.sync.dma_start(t[:], l1T.ap()[k * 128:(k + 1) * 128, :])
            l1_sb.append(t)
        l2_sb = []
        for k in range(4):
            t = const.tile([128, 256], f16, tag=f"l2_{k}")
            nc.sync.dma_start(t[:], l2T.ap()[k * 128:(k + 1) * 128, :])
            l2_sb.append(t)
        l3_sb = []
        for k in range(2):
            t = const.tile([128, 40], f16, tag=f"l3_{k}")
            nc.sync.dma_start(t[:], l3T.ap()[k * 128:(k + 1) * 128, :])
            l3_sb.append(t)

        ones_col = const.tile([128, 1], f32, tag="ones_col")
        nc.vector.memset(ones_col[:], 1.0)
        ones_row = const.tile([1, 128], f32, tag="ones_row")
        nc.vector.memset(ones_row[:], 1.0)
        ones16 = const.tile([1, 128], f16, tag="ones16")
        nc.vector.memset(ones16[:], 1.0)
        # f16 identity for PE-transpose of the topk index tiles (indices are
        # moved as exact f16 integer values; Ldweights only takes fp dtypes)
        ident = const.tile([128, 128], f16, tag="ident")
        make_identity(nc, ident[:])

        # dummy dma_gather at startup: pulls the Pool engine's SWDGE ucode
        # LIBRARY_RELOAD (~10us) off L1's first-gather critical path
        warm_idx = const.tile([16, 8], i16, tag="warm_idx")
        nc.vector.memset(warm_idx[:], 0)
        warm_out = const.tile([128, 128], f16, tag="warm_out")
        nc.gpsimd.dma_gather(
            warm_out[:].rearrange("p (f i) -> p f i", f=1),
            w5T.ap()[:, 0:128], warm_idx[:], 128, 128, 128,
            elem_step=1024, transpose=True, queue_num=0)

        # fp16 copies of the edge-conv outputs, consumed by the fp16 conv5.
        # Split into point-halves so conv5's half-h matmuls become ready as
        # soon as the producing layer's first/last 4 chunks finish (lets the
        # scheduler overlap conv5 with L4's DVE-bound topk phase).
        def half_tiles(rows, tag):
            return [xpool.tile([rows, 512], f16, tag=f"{tag}_{h}",
                               name=f"{tag}_{h}")
                    for h in range(2)]
        xh1 = half_tiles(64, "xh1")
        xh2 = half_tiles(64, "xh2")
        xh3 = half_tiles(128, "xh3")
        xh4a = half_tiles(128, "xh4a")
        xh4b = half_tiles(128, "xh4b")


        # ---------------- edge conv layer ----------------
        def edge_layer(li, xch, C, O, xouts):
            """xch: [C, 1024] channel-major SBUF AP.
            xouts: list of [rows, 1024] channel-major target tiles, one per
            128-channel block (block f holds channels f*128..)."""
            Opad = max(O, 128)       # gather element = Opad fp16 (>=256B)
            Of = Opad // 128

            # u^T rows (fp16) to DRAM as the gather source
            uTd = dram.tile([N, 256], f16, tag="uTd")
            u_src = uTd[:, 0:Opad]   # ap[0] step 256 -> elem_step

            # xx = colsum(x*x). For C<128 build xar = [x; -xx/2] so the
            # distance block is ONE matmul (lhsT = [x; ones], rhs = xar).
            aug = C < 128
            onesrow = 32 if C < 32 else C   # 32-aligned for L1 (C=3)
            Ka = onesrow + 1
            xsq = work.tile([C, 1024], f32, tag="xsq")
            for mq in range(8):
                qsl = slice(mq * 128, (mq + 1) * 128)
                nc.vector.tensor_mul(xsq[:, qsl], xch[0:C, qsl],
                                     xch[0:C, qsl])
            if aug:
                xar = work.tile([Ka, 1024], f32, tag="xar")
                if onesrow > C:
                    # rows C..onesrow pair with zero rows of xch in the
                    # augmented matmul; they must not hold NaN garbage
                    nc.vector.memset(xar[0:onesrow, :], 0.0)
                for mq in range(8):
                    qsl = slice(mq * 128, (mq + 1) * 128)
                    nc.scalar.activation(xar[0:C, qsl], xch[0:C, qsl],
                                         Act.Copy)
                nxx = xar[onesrow:Ka, :]
            else:
                nxxt = work.tile([1, 1024], f32, tag="nxx")
                nxx = nxxt[:]
            for h in range(2):
                ps = mm.tile([1, 512], f32, tag="mm")
                nc.tensor.matmul(ps[:], ones_col[0:C, :],
                    xsq[:, h * 512:(h + 1) * 512])
                nc.scalar.activation(nxx[0:1, h * 512:(h + 1) * 512], ps[:],
                                     Act.Copy, scale=-0.5)
            # u^T per 128-point chunk (point-major, fp16, with c0 folded in)
            for m in range(8):
                csl = slice(m * 128, (m + 1) * 128)
                pu = sm.tile([128, O], f32, tag="sm")
                nc.tensor.matmul(pu[:], xch[0:C, csl], AT_sb[li][:],
                    start=True, stop=False)
                nc.tensor.matmul(pu[:], ones_row[:], c0_sb[li][:],
                    start=False, stop=True)
                uT = work.tile([128, O], f16, tag="uT", bufs=3)
                nc.scalar.activation(uT[:], pu[:], Act.Copy)
                nc.scalar.dma_start(uTd[csl, 0:O], uT[:])

            # v channel-major [O, 1024] (f32)
            vs = []
            for f in range(Of if O >= 128 else 1):
                osl = slice(f * 128, min((f + 1) * 128, O))
                orows = osl.stop - osl.start
                vt = vt_p.tile([128, 1024], f32, tag="vt")
                for h in range(2):
                    nsl = slice(h * 512, (h + 1) * 512)
                    pv = mm.tile([128, 512], f32, tag="mm")
                    nc.tensor.matmul(pv[0:orows, :], BT_sb[li][:, osl],
                        xch[0:C, nsl])
                    nc.scalar.activation(vt[0:orows, nsl], pv[0:orows, :], Act.Copy)
                vs.append(vt)

            # gather pipeline: shuffle chunk m's idx rows, then its 4
            # queue-split gathers (emitted one chunk behind the topk loop so
            # Pool desc-gen and the DMA rings overlap the next topk)
            gq_tiles = {}

            def emit_gather(m):
                idxs = idx_tiles[m]
                gqs = []
                for q in range(4):
                    gq = gth_p.tile([128, Of * 640], f16, tag="gth")
                    nc.gpsimd.dma_gather(
                        gq[:].rearrange("p (f i) -> p f i", f=Of),
                        u_src,
                        idxs[:, q * 40:(q + 1) * 40],
                        640, 640, Opad, elem_step=256, transpose=True,
                        queue_num=q,
                    )
                    gqs.append(gq)
                gq_tiles[m] = gqs

            # neighbor max for chunk m (fp16 trees on the Pool engine, which
            # is otherwise idle between gather desc-gens), then
            # x_next = lrelu(mk + v) channel-major (DVE)
            def emit_tree(m):
                csl = slice(m * 128, (m + 1) * 128)
                mkT = small.tile([128, Of * 128], f16, tag="mkT")
                for f in range(Of):
                    ga, gb, gc, gd = (
                        g[:].rearrange("p (ff i) -> p ff i", ff=Of)[:, f, :]
                        for g in gq_tiles[m])
                    # cross-queue max first at full width (fewer, larger DVE
                    # ops: instruction overhead dominates [128,128] pieces)
                    tA = small.tile([128, 640], f16, tag="tA", bufs=2)
                    nc.vector.tensor_tensor(out=tA[:], in0=ga, in1=gb,
                                            op=Alu.max)
                    tB = small.tile([128, 640], f16, tag="tB", bufs=2)
                    nc.vector.tensor_tensor(out=tB[:], in0=gc, in1=gd,
                                            op=Alu.max)
                    tC = small.tile([128, 640], f16, tag="tC", bufs=2)
                    nc.vector.tensor_tensor(out=tC[:], in0=tA[:], in1=tB[:],
                                            op=Alu.max)
                    tD = small.tile([128, 256], f16, tag="tD", bufs=2)
                    nc.vector.tensor_tensor(out=tD[:], in0=tC[:, 0:256],
                                            in1=tC[:, 256:512], op=Alu.max)
                    tE = small.tile([128, 128], f16, tag="tE", bufs=2)
                    nc.vector.tensor_tensor(out=tE[:], in0=tD[:, 0:128],
                                            in1=tD[:, 128:256], op=Alu.max)
                    nc.vector.tensor_tensor(
                        out=mkT[:, f * 128:(f + 1) * 128], in0=tE[:],
                        in1=tC[:, 512:640], op=Alu.max)

                for f, (xt, xh, rows) in enumerate(xouts):
                    z = small.tile([128, 128], f32, tag="z")
                    nc.vector.tensor_add(z[0:rows, :],
                                         mkT[0:rows, f * 128:f * 128 + 128],
                                         vs[f][0:rows, csl])
                    nc.vector.scalar_tensor_tensor(
                        out=xt[0:rows, csl], in0=z[0:rows, :], scalar=SLOPE,
                        in1=z[0:rows, :], op0=Alu.mult, op1=Alu.max)
                    # fp16 copy for conv5 (ACT engine, off the DVE path)
                    lsl = slice((m % 4) * 128, (m % 4) * 128 + 128)
                    nc.scalar.activation(xh[m // 4][0:rows, lsl],
                                         xt[0:rows, csl], Act.Copy)

            # ---- phase B: dist + topk + idx wrap, all 8 chunks, with the
            # previous chunk's gather and the chunk-before-that's tree
            # emitted one/two behind so every engine pipelines ----
            idx_tiles = []
            for m in range(8):
                csl = slice(m * 128, (m + 1) * 128)
                srow = srow_p.tile([128, 1024], f32, tag="srow")
                for h in range(2):
                    nsl = slice(h * 512, (h + 1) * 512)
                    pd = mm.tile([128, 512], f32, tag="mm")
                    if aug:
                        nc.tensor.matmul(pd[:], xch[0:Ka, csl],
                            xar[:, nsl])
                    else:
                        nc.tensor.matmul(pd[:], xch[0:C, csl], xch[0:C, nsl],
                            start=True, stop=False)
                        nc.tensor.matmul(pd[:], ones_row[:], nxx[0:1, nsl],
                            start=False, stop=True)
                    nc.scalar.activation(srow[:, nsl], pd[:], Act.Copy)

                idx20 = small.tile([128, 20], u16, tag="idx20", bufs=6)
                idx8b = small.tile([128, 8], u16, tag="idx8b")
                for r in range(3):
                    m8 = small.tile([128, 8], f32, tag="m8")
                    nc.vector.max(m8[:], srow[:])
                    dst8 = idx20[:, r * 8:r * 8 + 8] if r < 2 else idx8b[:]
                    nc.vector.max_index(dst8, m8[:], srow[:])
                    if r < 2:
                        nc.vector.match_replace(srow[:], m8[:], srow[:], NEG)
                nc.vector.tensor_copy(idx20[:, 16:20], idx8b[:, 0:4])

                # Gather flat order is c-major: flat[i] = idx[point i%128]
                # [nbr i//128]; wrapped layout sbuf[q, s] = flat[s*16+q] per
                # 32-partition quadrant. Affine in (t=p//16, q=p%16, c):
                # F2W[q*160 + c*8 + t] = idx20[t*16+q, c].
                # Build it via PE transpose (u16 pass-through) + a free-dim
                # permute copy, so the DRAM write is 16B-contiguous runs
                # instead of scattered 2B granules.
                idxf = work.tile([128, 20], f16, tag="idxf", bufs=4)
                nc.vector.tensor_copy(idxf[:], idx20[:])
                # allocate from the dist-matmul PSUM tag: the round-robin
                # buffer rotation stops PE from running dist matmuls more
                # than ~2 chunks ahead of the transposes they'd starve
                tpp = mm.tile([20, 128], f16, tag="mm", name="tpp")
                # high priority: the transpose gates the whole gather path
                # and must not queue behind the run-ahead dist matmuls on PE
                with tc.high_priority():
                    nc.tensor.transpose(tpp[:], idxf[:], ident[:])
                tcw = work.tile([20, 128], u16, tag="tcw", bufs=4)
                nc.vector.tensor_copy(
                    tcw[:].rearrange("c (q t) -> c t q", q=16, t=8),
                    tpp[:])
                F2W = dram_s.tile([2560], u16, tag="F2")
                w4 = F2W[:].rearrange("(q c t) -> c q t", q=16, c=20, t=8)
                nc.sync.dma_start(w4, tcw[:])
                # read back replicated into all 8 16-partition groups (the
                # layout dma_gather's SWDGE descriptor generator expects)
                idxw = work.tile([128, 160], i16, tag="idxw", bufs=9)
                rdq = F2W[:].bitcast(i16).rearrange("(q s) -> q s", q=16)
                for r in range(8):
                    nc.sync.dma_start(idxw[16 * r:16 * r + 16, :], rdq)
                idx_tiles.append(idxw)
                emit_gather(m)
                lag = 5
                if m >= lag:
                    emit_tree(m - lag)
            for mm_ in range(8 - lag, 8):
                emit_tree(mm_)

        edge_layer(0, x0[:], 3, 64, [(x1, xh1, 64)])
        edge_layer(1, x1[:], 64, 64, [(x2, xh2, 64)])
        edge_layer(2, x2[:], 64, 128, [(x3, xh3, 128)])
        edge_layer(3, x3[:], 128, 256, [(x4a, xh4a, 128), (x4b, xh4b, 128)])

        # ---------------- conv5 (512 -> 1024) + global max pool ----------------
        # conv5 PSUM banks are drained by ACT (fp16 evac) so the PE can pack
        # conv5 groups into L4's idle stretches without waiting on the busy
        # DVE; the global max pool then runs as 8 cheap rowmax ops at the end.
        xc_full = [(xh1, 64), (xh2, 64), (xh3, 128), (xh4a, 128), (xh4b, 128)]
        e_sb = []
        for mo in range(8):
            msl = slice(mo * 128, (mo + 1) * 128)
            et = xpool.tile([128, 1024], f16, tag=f"e{mo}", name=f"e{mo}")
            for h in range(2):
                pe = mm.tile([128, 512], f32, tag="mm")
                for k in range(5):
                    nc.tensor.matmul(pe[:], w5_sb[k][:, msl],
                        xc_full[k][0][h][0:xc_full[k][1], :],
                        start=(k == 0), stop=(k == 4))
                nc.scalar.activation(et[:, h * 512:(h + 1) * 512], pe[:],
                                     Act.Copy)
            e_sb.append(et)
        for mo in range(8):
            nc.vector.reduce_max(gp[:, mo:mo + 1], e_sb[mo][:], axis=AX.X)

        # ---------------- MLP head (fp16 weights/activations) ----------------
        gph = small.tile([128, 8], f16, tag="gph")
        nc.scalar.activation(gph[:], gp[:], Act.Copy)
        y1 = small.tile([128, 4], f16, tag="y1")
        for mt in range(4):
            msl = slice(mt * 128, (mt + 1) * 128)
            p1 = sm.tile([128, 1], f32, tag="sm")
            for k in range(8):
                nc.tensor.matmul(p1[:], l1_sb[k][:, msl], gph[:, k:k + 1],
                    start=(k == 0), stop=False)
            nc.tensor.matmul(p1[:], b6_sb[0:1, msl], ones16[0:1, 0:1],
                start=False, stop=True)
            t1 = small.tile([128, 1], f32, tag="t1")
            nc.scalar.activation(t1[:], p1[:], Act.Copy)
            nc.vector.scalar_tensor_tensor(
                out=y1[:, mt:mt + 1], in0=t1[:], scalar=SLOPE, in1=t1[:],
                op0=Alu.mult, op1=Alu.max)

        y2 = small.tile([128, 2], f16, tag="y2")
        for mt in range(2):
            msl = slice(mt * 128, (mt + 1) * 128)
            p2 = sm.tile([128, 1], f32, tag="sm")
            for k in range(4):
                nc.tensor.matmul(p2[:], l2_sb[k][:, msl], y1[:, k:k + 1],
                    start=(k == 0), stop=False)
            nc.tensor.matmul(p2[:], c7_sb[0:1, msl], ones16[0:1, 0:1],
                start=False, stop=True)
            t2 = small.tile([128, 1], f32, tag="t2")
            nc.scalar.activation(t2[:], p2[:], Act.Copy)
            nc.vector.scalar_tensor_tensor(
                out=y2[:, mt:mt + 1], in0=t2[:], scalar=SLOPE, in1=t2[:],
                op0=Alu.mult, op1=Alu.max)

        p3 = sm.tile([40, 1], f32, tag="sm")
        for k in range(2):
            nc.tensor.matmul(p3[:], l3_sb[k][:, 0:40], y2[:, k:k + 1],
                start=(k == 0), stop=False)
        nc.tensor.matmul(p3[:], b8_sb[0:1, 0:40], ones16[0:1, 0:1],
            start=False, stop=True)
        y3 = small.tile([40, 1], f32, tag="y3")
        nc.scalar.activation(y3[:], p3[:], Act.Copy)
        nc.sync.dma_start(out_d.ap(), y3[:])

    nc.compile()
    return nc


def _prep_inputs(inputs):
    """Fold eval-mode BN into conv/linear weights; transpose for the device."""
    f = np.float32
    s = lambda g: (g / np.sqrt(f(1.0) + f(EPS))).astype(f)

    def edge(w, g, b, bias=None):
        O, C2 = w.shape
        C = C2 // 2
        sc = s(g)
        Wd = w[:, :C]
        Wc = w[:, C:]
        A = sc[:, None] * Wd
        Bm = sc[:, None] * (Wc - Wd)
        c0 = sc * (bias if bias is not None else 0.0) + b
        return A.T.copy().astype(f), Bm.T.copy().astype(f), c0.reshape(1, -1).astype(f)

    d = {}
    d["AT0"], d["BT0"], d["c00"] = edge(inputs["conv1_w"], inputs["bn1_g"],
                                        inputs["bn1_b"], inputs["conv1_b"])
    d["AT1"], d["BT1"], d["c01"] = edge(inputs["conv2_w"], inputs["bn2_g"], inputs["bn2_b"])
    d["AT2"], d["BT2"], d["c02"] = edge(inputs["conv3_w"], inputs["bn3_g"], inputs["bn3_b"])
    d["AT3"], d["BT3"], d["c03"] = edge(inputs["conv4_w"], inputs["bn4_g"], inputs["bn4_b"])
    h = np.float16
    d["w5T"] = inputs["conv5_w"].T.copy().astype(h)
    s6 = s(inputs["bn6_g"])
    d["l1T"] = (s6[:, None] * inputs["lin1_w"]).T.copy().astype(h)
    d["b6"] = inputs["bn6_b"].reshape(1, -1).astype(h)
    s7 = s(inputs["bn7_g"])
    d["l2T"] = (s7[:, None] * inputs["lin2_w"]).T.copy().astype(h)
    d["c7"] = (s7 * inputs["lin2_b"] + inputs["bn7_b"]).reshape(1, -1).astype(h)
    d["l3T"] = inputs["lin3_w"].T.copy().astype(h)
    d["b8"] = inputs["lin3_b"].reshape(1, -1).astype(h)
    return d


def _install_ntff_hook():
    """The agent image's antenv lacks axon_hooks; synthesize it and register
    the ctypes NTFF profiling hook from trn_agent_boot (same as trn_boot)."""
    import sys
    import types

    if "antenv.axon_hooks" in sys.modules:
        return
    import antenv

    mod = types.ModuleType("antenv.axon_hooks")
    holder = [None]
    mod.set_axon_ntff_profile_hook = lambda h: holder.__setitem__(0, h)
    mod.get_axon_ntff_profile_hook = lambda: holder[0]
    sys.modules["antenv.axon_hooks"] = mod
    antenv.axon_hooks = mod
    try:
        from trn_agent_boot.trn_boot import _ntff_profile_via_ctypes

        mod.set_axon_ntff_profile_hook(
            _ntff_profile_via_ctypes("/opt/axon/libaxon_pjrt.so"))
    except Exception as e:
        print(f"NTFF hook install failed: {e}")


def kernel(**inputs):
    global LAST_RESULTS
    from concourse.bass_utils import run_bass_kernel_spmd

    if "nc" not in _CACHE:
        _CACHE["nc"] = _build()
    nc = _CACHE["nc"]

    x = np.asarray(inputs["x"], dtype=np.float32)  # (8, 1024, 3)
    common = _prep_inputs({k: np.asarray(v) for k, v in inputs.items()})
    in_maps = [dict(common, xin=np.ascontiguousarray(x[i].T)) for i in range(NCORES)]

    trace = bool(int(os.environ.get("DGCNN_TRACE", "0")))
    if trace:
        _install_ntff_hook()
    res = run_bass_kernel_spmd(nc, in_maps, core_ids=list(range(NCORES)),
                               trace=trace, trace_cores=[0] if trace else None)
    LAST_RESULTS = res
    out = np.stack([r["out"].reshape(40) for r in res.results]).astype(np.float32)
    return out



# revision 53
# speedup vs baseline: 1.5374x; 1.5374x over previous
"""DGCNN forward on 8 Trainium2 NeuronCores (Bass/Tile), pure data parallel.

Each core processes one sample (N=1024 points, K=20 neighbors).

Algorithmic mapping per EdgeConv layer (weights BN-folded on host):
  y[:,n,j] = Wd@(x_nbr - x_ctr) + Wc@x_ctr   (1x1 conv on edge features)
           = Wd@x[:,idx[n,j]] + (Wc-Wd)@x[:,n]
  After folding the (eval-mode) BN scale s and bias into the weights, and
  because max over neighbors commutes with the monotone LeakyReLU:
  out[:,n] = lrelu( max_j u[:,idx[n,j]] + v[:,n] )
  with u = (s*Wd)@x + (s*bias + b)  and  v = (s*(Wc-Wd))@x.

  KNN row scores: top-20 of  s[n,m] = <x_n,x_m> - ||x_m||^2/2  (equivalent
  ordering to the reference's -||x_n-x_m||^2 per row).

Top-20 per row: 3 rounds of DVE max8 / max_index / match_replace.
Neighbor gather: gpsimd dma_gather of u^T rows from DRAM, split over the
4 SWDGE queues; reduce-max over the 20 gathered rows on the Pool engine.
"""

import os

import numpy as np

N = 1024
K = 20
NCORES = 8
EPS = 1e-5
SLOPE = 0.01
NEG = -3.0e38

# (C_in, O) per edge conv layer
EDGE_LAYERS = [(3, 64), (64, 64), (64, 128), (128, 256)]

_CACHE = {}
LAST_RESULTS = None


def _build():
    import concourse.bass as bass
    import concourse.mybir as mybir
    import concourse.tile as tile
    from concourse import bacc

    dt = mybir.dt
    f32 = dt.float32
    u16 = dt.uint16
    f16 = dt.float16
    i16 = dt.int16
    Alu = mybir.AluOpType
    Act = mybir.ActivationFunctionType
    AX = mybir.AxisListType
    f32r = dt.float32r

    nc = bacc.Bacc("TRN2", target_bir_lowering=False, debug=False,
                   num_swdge_queues=4)
    from concourse.masks import make_identity

    def mmr(out, lhsT, rhs, **kw):
        # fp32 matmul at full PE rate via the float32r replication mode
        nc.tensor.matmul(out, lhsT.bitcast(f32r), rhs.bitcast(f32r), **kw)

    # ---------------- DRAM I/O ----------------
    xin = nc.dram_tensor("xin", [3, N], f32, kind="ExternalInput")
    ATs, BTs, c0s = [], [], []
    for li, (C, O) in enumerate(EDGE_LAYERS):
        ATs.append(nc.dram_tensor(f"AT{li}", [C, O], f32, kind="ExternalInput"))
        BTs.append(nc.dram_tensor(f"BT{li}", [C, O], f32, kind="ExternalInput"))
        c0s.append(nc.dram_tensor(f"c0{li}", [1, O], f32, kind="ExternalInput"))
    # conv5 + MLP head run in fp16 (full PE rate); weights cast on host
    w5T = nc.dram_tensor("w5T", [512, 1024], f16, kind="ExternalInput")
    l1T = nc.dram_tensor("l1T", [1024, 512], f16, kind="ExternalInput")
    b6 = nc.dram_tensor("b6", [1, 512], f16, kind="ExternalInput")
    l2T = nc.dram_tensor("l2T", [512, 256], f16, kind="ExternalInput")
    c7 = nc.dram_tensor("c7", [1, 256], f16, kind="ExternalInput")
    l3T = nc.dram_tensor("l3T", [256, 40], f16, kind="ExternalInput")
    b8 = nc.dram_tensor("b8", [1, 40], f16, kind="ExternalInput")
    out_d = nc.dram_tensor("out", [40, 1], f32, kind="ExternalOutput")

    with tile.TileContext(nc) as tc, __import__("contextlib").ExitStack() as ctx:
        const = ctx.enter_context(tc.tile_pool(name="const", bufs=1))
        xpool = ctx.enter_context(tc.tile_pool(name="xpool", bufs=1))
        work = ctx.enter_context(tc.tile_pool(name="work", bufs=2))
        srow_p = ctx.enter_context(tc.tile_pool(name="srow", bufs=4))
        gth_p = ctx.enter_context(tc.tile_pool(name="gth", bufs=20))
        vt_p = ctx.enter_context(tc.tile_pool(name="vt", bufs=3))
        small = ctx.enter_context(tc.tile_pool(name="small", bufs=4))
        mm = ctx.enter_context(tc.tile_pool(name="mm", bufs=4, space="PSUM"))
        sm = ctx.enter_context(tc.tile_pool(name="sm", bufs=3, space="PSUM"))
        tp = ctx.enter_context(tc.tile_pool(name="tp", bufs=1, space="PSUM"))
        dram = ctx.enter_context(tc.tile_pool(name="dram", bufs=2, space="DRAM"))
        dram_s = ctx.enter_context(tc.tile_pool(name="dram_s", bufs=3, space="DRAM"))

        # persistent channel-major feature tiles; layers whose KNN input has
        # C<128 carry an extra all-ones channel row used to fold the -xx/2
        # rank-1 term into the single distance matmul (lhsT side).
        # x0 is padded to 33 rows: x in rows 0:3, zeros in 3:32 (so they
        # contribute nothing to the K=33 distance matmul), ones row at 32
        # (engine writes must start at a 32-aligned partition).
        x0 = xpool.tile([33, 1024], f32, tag="x0")
        x1 = xpool.tile([65, 1024], f32, tag="x1")
        x2 = xpool.tile([65, 1024], f32, tag="x2")
        x3 = xpool.tile([128, 1024], f32, tag="x3")
        x4a = xpool.tile([128, 1024], f32, tag="x4a")
        x4b = xpool.tile([128, 1024], f32, tag="x4b")
        gp = xpool.tile([128, 8], f32, tag="gp")
        nc.vector.memset(x0[:], 0.0)
        nc.vector.memset(x0[32:33, :], 1.0)
        nc.vector.memset(x1[64:65, :], 1.0)
        nc.vector.memset(x2[64:65, :], 1.0)

        # load x (host pre-transposed channel-major [3, 1024]; a device-side
        # transposing DMA would be 4B-granule descriptor hell) FIRST so L1
        # isn't blocked behind the ~3MB of weight loads below.
        nc.sync.dma_start(x0[0:3, :], xin.ap())

        # ------------- constants into SBUF -------------
        def load_const(name, dram_t, shape=None, dtype=None):
            t = const.tile(list(shape or dram_t.shape), dtype or f32, tag=name)
            nc.sync.dma_start(t[:], dram_t.ap())
            return t

        AT_sb = [load_const(f"AT{i}", ATs[i]) for i in range(4)]
        BT_sb = [load_const(f"BT{i}", BTs[i]) for i in range(4)]
        c0_sb = [load_const(f"c0{i}", c0s[i]) for i in range(4)]
        b6_sb = load_const("b6", b6, dtype=f16)
        c7_sb = load_const("c7", c7, dtype=f16)
        b8_sb = load_const("b8", b8, dtype=f16)

        # w5T: 5 K-chunks matching [x1(64), x2(64), x3(128), x4a(128), x4b(128)]
        # fp16: conv5 + head run at full PE rate
        w5_rows = [(0, 64), (64, 128), (128, 256), (256, 384), (384, 512)]
        w5_sb = []
        for i, (r0, r1) in enumerate(w5_rows):
            t = const.tile([r1 - r0, 1024], f16, tag=f"w5_{i}")
            nc.sync.dma_start(t[:], w5T.ap()[r0:r1, :])
            w5_sb.append(t)
        l1_sb = []
        for k in range(8):
            t = const.tile([128, 512], f16, tag=f"l1_{k}")
            nc.sync.dma_start(t[:], l1T.ap()[k * 128:(k + 1) * 128, :])
            l1_sb.append(t)
        l2_sb = []
        for k in range(4):
            t = const.tile([128, 256], f16, tag=f"l2_{k}")
            nc.sync.dma_start(t[:], l2T.ap()[k * 128:(k + 1) * 128, :])
            l2_sb.append(t)
        l3_sb = []
        for k in range(2):
            t = const.tile([128, 40], f16, tag=f"l3_{k}")
            nc.sync.dma_start(t[:], l3T.ap()[k * 128:(k + 1) * 128, :])
            l3_sb.append(t)

        ones_col = const.tile([128, 1], f32, tag="ones_col")
        nc.vector.memset(ones_col[:], 1.0)
        ones_row = const.tile([1, 128], f32, tag="ones_row")
        nc.vector.memset(ones_row[:], 1.0)
        ones16 = const.tile([1, 128], f16, tag="ones16")
        nc.vector.memset(ones16[:], 1.0)
        # f16 identity for PE-transpose of the topk index tiles (indices are
        # moved as exact f16 integer values; Ldweights only takes fp dtypes)
        ident = const.tile([128, 128], f16, tag="ident")
        make_identity(nc, ident[:])

        # dummy dma_gather at startup: pulls the Pool engine's SWDGE ucode
        # LIBRARY_RELOAD (~10us) off L1's first-gather critical path
        warm_idx = const.tile([16, 8], i16, tag="warm_idx")
        nc.vector.memset(warm_idx[:], 0)
        warm_out = const.tile([128, 128], f16, tag="warm_out")
        nc.gpsimd.dma_gather(
            warm_out[:].rearrange("p (f i) -> p f i", f=1),
            w5T.ap()[:, 0:128], warm_idx[:], 128, 128, 128,
            elem_step=1024, transpose=True, queue_num=0)

        # fp16 copies of the edge-conv outputs, consumed by the fp16 conv5.
        # Split into point-halves so conv5's half-h matmuls become ready as
        # soon as the producing layer's first/last 4 chunks finish (lets the
        # scheduler overlap conv5 with L4's DVE-bound topk phase).
        def half_tiles(rows, tag):
            return [xpool.tile([rows, 512], f16, tag=f"{tag}_{h}",
                               name=f"{tag}_{h}")
                    for h in range(2)]
        xh1 = half_tiles(64, "xh1")
        xh2 = half_tiles(64, "xh2")
        xh3 = half_tiles(128, "xh3")
        xh4a = half_tiles(128, "xh4a")
        xh4b = half_tiles(128, "xh4b")


        # ---------------- edge conv layer ----------------
        def edge_layer(li, xch, C, O, xouts):
            """xch: [C, 1024] channel-major SBUF AP.
            xouts: list of [rows, 1024] channel-major target tiles, one per
            128-channel block (block f holds channels f*128..)."""
            Opad = max(O, 128)       # gather element = Opad fp16 (>=256B)
            Of = Opad // 128

            # u^T rows (fp16) to DRAM as the gather source
            uTd = dram.tile([N, 256], f16, tag="uTd")
            u_src = uTd[:, 0:Opad]   # ap[0] step 256 -> elem_step

            # xx = colsum(x*x). For C<128 build xar = [x; -xx/2] so the
            # distance block is ONE matmul (lhsT = [x; ones], rhs = xar).
            aug = C < 128
            onesrow = 32 if C < 32 else C   # 32-aligned for L1 (C=3)
            Ka = onesrow + 1
            xsq = work.tile([C, 1024], f32, tag="xsq")
            for mq in range(8):
                qsl = slice(mq * 128, (mq + 1) * 128)
                nc.vector.tensor_mul(xsq[:, qsl], xch[0:C, qsl],
                                     xch[0:C, qsl])
            if aug:
                xar = work.tile([Ka, 1024], f32, tag="xar")
                if onesrow > C:
                    # rows C..onesrow pair with zero rows of xch in the
                    # augmented matmul; they must not hold NaN garbage
                    nc.vector.memset(xar[0:onesrow, :], 0.0)
                for mq in range(8):
                    qsl = slice(mq * 128, (mq + 1) * 128)
                    nc.scalar.activation(xar[0:C, qsl], xch[0:C, qsl],
                                         Act.Copy)
                nxx = xar[onesrow:Ka, :]
            else:
                nxxt = work.tile([1, 1024], f32, tag="nxx")
                nxx = nxxt[:]
            for h in range(2):
                ps = mm.tile([1, 512], f32, tag="mm")
                nc.tensor.matmul(ps[:], ones_col[0:C, :],
                    xsq[:, h * 512:(h + 1) * 512])
                nc.scalar.activation(nxx[0:1, h * 512:(h + 1) * 512], ps[:],
                                     Act.Copy, scale=-0.5)
            # u^T per 128-point chunk (point-major, fp16, with c0 folded in)
            for m in range(8):
                csl = slice(m * 128, (m + 1) * 128)
                pu = sm.tile([128, O], f32, tag="sm")
                nc.tensor.matmul(pu[:], xch[0:C, csl], AT_sb[li][:],
                    start=True, stop=False)
                nc.tensor.matmul(pu[:], ones_row[:], c0_sb[li][:],
                    start=False, stop=True)
                uT = work.tile([128, O], f16, tag="uT", bufs=3)
                nc.scalar.activation(uT[:], pu[:], Act.Copy)
                nc.scalar.dma_start(uTd[csl, 0:O], uT[:])

            # v channel-major [O, 1024] (f32)
            vs = []
            for f in range(Of if O >= 128 else 1):
                osl = slice(f * 128, min((f + 1) * 128, O))
                orows = osl.stop - osl.start
                vt = vt_p.tile([128, 1024], f32, tag="vt")
                for h in range(2):
                    nsl = slice(h * 512, (h + 1) * 512)
                    pv = mm.tile([128, 512], f32, tag="mm")
                    nc.tensor.matmul(pv[0:orows, :], BT_sb[li][:, osl],
                        xch[0:C, nsl])
                    nc.scalar.activation(vt[0:orows, nsl], pv[0:orows, :], Act.Copy)
                vs.append(vt)

            # gather pipeline: shuffle chunk m's idx rows, then its 4
            # queue-split gathers (emitted one chunk behind the topk loop so
            # Pool desc-gen and the DMA rings overlap the next topk)
            gq_tiles = {}

            def emit_gather(m):
                idxs = idx_tiles[m]
                gqs = []
                for q in range(4):
                    gq = gth_p.tile([128, Of * 640], f16, tag="gth")
                    nc.gpsimd.dma_gather(
                        gq[:].rearrange("p (f i) -> p f i", f=Of),
                        u_src,
                        idxs[:, q * 40:(q + 1) * 40],
                        640, 640, Opad, elem_step=256, transpose=True,
                        queue_num=q,
                    )
                    gqs.append(gq)
                gq_tiles[m] = gqs

            # neighbor max for chunk m (fp16 trees on the Pool engine, which
            # is otherwise idle between gather desc-gens), then
            # x_next = lrelu(mk + v) channel-major (DVE)
            def emit_tree(m):
                csl = slice(m * 128, (m + 1) * 128)
                mkT = small.tile([128, Of * 128], f16, tag="mkT")
                for f in range(Of):
                    ga, gb, gc, gd = (
                        g[:].rearrange("p (ff i) -> p ff i", ff=Of)[:, f, :]
                        for g in gq_tiles[m])
                    # cross-queue max first at full width (fewer, larger DVE
                    # ops: instruction overhead dominates [128,128] pieces)
                    tA = small.tile([128, 640], f16, tag="tA", bufs=2)
                    nc.vector.tensor_tensor(out=tA[:], in0=ga, in1=gb,
                                            op=Alu.max)
                    tB = small.tile([128, 640], f16, tag="tB", bufs=2)
                    nc.vector.tensor_tensor(out=tB[:], in0=gc, in1=gd,
                                            op=Alu.max)
                    tC = small.tile([128, 640], f16, tag="tC", bufs=2)
                    nc.vector.tensor_tensor(out=tC[:], in0=tA[:], in1=tB[:],
                                            op=Alu.max)
                    tD = small.tile([128, 256], f16, tag="tD", bufs=2)
                    nc.vector.tensor_tensor(out=tD[:], in0=tC[:, 0:256],
                                            in1=tC[:, 256:512], op=Alu.max)
                    tE = small.tile([128, 128], f16, tag="tE", bufs=2)
                    nc.vector.tensor_tensor(out=tE[:], in0=tD[:, 0:128],
                                            in1=tD[:, 128:256], op=Alu.max)
                    nc.vector.tensor_tensor(
                        out=mkT[:, f * 128:(f + 1) * 128], in0=tE[:],
                        in1=tC[:, 512:640], op=Alu.max)

                for f, (xt, xh, rows) in enumerate(xouts):
                    z = small.tile([128, 128], f32, tag="z")
                    nc.vector.tensor_add(z[0:rows, :],
                                         mkT[0:rows, f * 128:f * 128 + 128],
                                         vs[f][0:rows, csl])
                    nc.vector.scalar_tensor_tensor(
                        out=xt[0:rows, csl], in0=z[0:rows, :], scalar=SLOPE,
                        in1=z[0:rows, :], op0=Alu.mult, op1=Alu.max)
                    # fp16 copy for conv5 (ACT engine, off the DVE path)
                    lsl = slice((m % 4) * 128, (m % 4) * 128 + 128)
                    nc.scalar.activation(xh[m // 4][0:rows, lsl],
                                         xt[0:rows, csl], Act.Copy)

            # ---- phase B: dist + topk + idx wrap, all 8 chunks, with the
            # previous chunk's gather and the chunk-before-that's tree
            # emitted one/two behind so every engine pipelines ----
            idx_tiles = []
            for m in range(8):
                csl = slice(m * 128, (m + 1) * 128)
                srow = srow_p.tile([128, 1024], f32, tag="srow")
                for h in range(2):
                    nsl = slice(h * 512, (h + 1) * 512)
                    pd = mm.tile([128, 512], f32, tag="mm")
                    if aug:
                        nc.tensor.matmul(pd[:], xch[0:Ka, csl],
                            xar[:, nsl])
                    else:
                        nc.tensor.matmul(pd[:], xch[0:C, csl], xch[0:C, nsl],
                            start=True, stop=False)
                        nc.tensor.matmul(pd[:], ones_row[:], nxx[0:1, nsl],
                            start=False, stop=True)
                    nc.scalar.activation(srow[:, nsl], pd[:], Act.Copy)

                idx20 = small.tile([128, 20], u16, tag="idx20", bufs=6)
                idx8b = small.tile([128, 8], u16, tag="idx8b")
                for r in range(3):
                    m8 = small.tile([128, 8], f32, tag="m8")
                    nc.vector.max(m8[:], srow[:])
                    dst8 = idx20[:, r * 8:r * 8 + 8] if r < 2 else idx8b[:]
                    nc.vector.max_index(dst8, m8[:], srow[:])
                    if r < 2:
                        nc.vector.match_replace(srow[:], m8[:], srow[:], NEG)
                nc.vector.tensor_copy(idx20[:, 16:20], idx8b[:, 0:4])

                # Gather flat order is c-major: flat[i] = idx[point i%128]
                # [nbr i//128]; wrapped layout sbuf[q, s] = flat[s*16+q] per
                # 32-partition quadrant. Affine in (t=p//16, q=p%16, c):
                # F2W[q*160 + c*8 + t] = idx20[t*16+q, c].
                # Build it via PE transpose (u16 pass-through) + a free-dim
                # permute copy, so the DRAM write is 16B-contiguous runs
                # instead of scattered 2B granules.
                idxf = work.tile([128, 20], f16, tag="idxf", bufs=4)
                nc.vector.tensor_copy(idxf[:], idx20[:])
                tpp = tp.tile([20, 128], f16, tag="tp")
                # high priority: the transpose gates the whole gather path
                # and must not queue behind the run-ahead dist matmuls on PE
                with tc.high_priority():
                    nc.tensor.transpose(tpp[:], idxf[:], ident[:])
                tcw = work.tile([20, 128], u16, tag="tcw", bufs=4)
                nc.vector.tensor_copy(
                    tcw[:].rearrange("c (q t) -> c t q", q=16, t=8),
                    tpp[:])
                F2W = dram_s.tile([2560], u16, tag="F2")
                w4 = F2W[:].rearrange("(q c t) -> c q t", q=16, c=20, t=8)
                nc.sync.dma_start(w4, tcw[:])
                # read back replicated into all 8 16-partition groups (the
                # layout dma_gather's SWDGE descriptor generator expects)
                idxw = work.tile([128, 160], i16, tag="idxw", bufs=9)
                rdq = F2W[:].bitcast(i16).rearrange("(q s) -> q s", q=16)
                for r in range(8):
                    nc.sync.dma_start(idxw[16 * r:16 * r + 16, :], rdq)
                idx_tiles.append(idxw)
                emit_gather(m)
                lag = 5
                if m >= lag:
                    emit_tree(m - lag)
            for mm_ in range(8 - lag, 8):
                emit_tree(mm_)

        edge_layer(0, x0[:], 3, 64, [(x1, xh1, 64)])
        edge_layer(1, x1[:], 64, 64, [(x2, xh2, 64)])
        edge_layer(2, x2[:], 64, 128, [(x3, xh3, 128)])
        edge_layer(3, x3[:], 128, 256, [(x4a, xh4a, 128), (x4b, xh4b, 128)])

        # ---------------- conv5 (512 -> 1024) + global max pool ----------------
        # conv5 PSUM banks are drained by ACT (fp16 evac) so the PE can pack
        # conv5 groups into L4's idle stretches without waiting on the busy
        # DVE; the global max pool then runs as 8 cheap rowmax ops at the end.
        xc_full = [(xh1, 64), (xh2, 64), (xh3, 128), (xh4a, 128), (xh4b, 128)]
        e_sb = []
        for mo in range(8):
            msl = slice(mo * 128, (mo + 1) * 128)
            et = xpool.tile([128, 1024], f16, tag=f"e{mo}", name=f"e{mo}")
            for h in range(2):
                pe = mm.tile([128, 512], f32, tag="mm")
                for k in range(5):
                    nc.tensor.matmul(pe[:], w5_sb[k][:, msl],
                        xc_full[k][0][h][0:xc_full[k][1], :],
                        start=(k == 0), stop=(k == 4))
                nc.scalar.activation(et[:, h * 512:(h + 1) * 512], pe[:],
                                     Act.Copy)
            e_sb.append(et)
        for mo in range(8):
            nc.vector.reduce_max(gp[:, mo:mo + 1], e_sb[mo][:], axis=AX.X)

        # ---------------- MLP head (fp16 weights/activations) ----------------
        gph = small.tile([128, 8], f16, tag="gph")
        nc.scalar.activation(gph[:], gp[:], Act.Copy)
        y1 = small.tile([128, 4], f16, tag="y1")
        for mt in range(4):
            msl = slice(mt * 128, (mt + 1) * 128)
            p1 = sm.tile([128, 1], f32, tag="sm")
            for k in range(8):
                nc.tensor.matmul(p1[:], l1_sb[k][:, msl], gph[:, k:k + 1],
                    start=(k == 0), stop=False)
            nc.tensor.matmul(p1[:], b6_sb[0:1, msl], ones16[0:1, 0:1],
                start=False, stop=True)
            t1 = small.tile([128, 1], f32, tag="t1")
            nc.scalar.activation(t1[:], p1[:], Act.Copy)
            nc.vector.scalar_tensor_tensor(
                out=y1[:, mt:mt + 1], in0=t1[:], scalar=SLOPE, in1=t1[:],
                op0=Alu.mult, op1=Alu.max)

        y2 = small.tile([128, 2], f16, tag="y2")
        for mt in range(2):
            msl = slice(mt * 128, (mt + 1) * 128)
            p2 = sm.tile([128, 1], f32, tag="sm")
            for k in range(4):
                nc.tensor.matmul(p2[:], l2_sb[k][:, msl], y1[:, k:k + 1],
                    start=(k == 0), stop=False)
            nc.tensor.matmul(p2[:], c7_sb[0:1, msl], ones16[0:1, 0:1],
                start=False, stop=True)
            t2 = small.tile([128, 1], f32, tag="t2")
            nc.scalar.activation(t2[:], p2[:], Act.Copy)
            nc.vector.scalar_tensor_tensor(
                out=y2[:, mt:mt + 1], in0=t2[:], scalar=SLOPE, in1=t2[:],
                op0=Alu.mult, op1=Alu.max)

        p3 = sm.tile([40, 1], f32, tag="sm")
        for k in range(2):
            nc.tensor.matmul(p3[:], l3_sb[k][:, 0:40], y2[:, k:k + 1],
                start=(k == 0), stop=False)
        nc.tensor.matmul(p3[:], b8_sb[0:1, 0:40], ones16[0:1, 0:1],
            start=False, stop=True)
        y3 = small.tile([40, 1], f32, tag="y3")
        nc.scalar.activation(y3[:], p3[:], Act.Copy)
        nc.sync.dma_start(out_d.ap(), y3[:])

    nc.compile()
    return nc


def _prep_inputs(inputs):
    """Fold eval-mode BN into conv/linear weights; transpose for the device."""
    f = np.float32
    s = lambda g: (g / np.sqrt(f(1.0) + f(EPS))).astype(f)

    def edge(w, g, b, bias=None):
        O, C2 = w.shape
        C = C2 // 2
        sc = s(g)
        Wd = w[:, :C]
        Wc = w[:, C:]
        A = sc[:, None] * Wd
        Bm = sc[:, None] * (Wc - Wd)
        c0 = sc * (bias if bias is not None else 0.0) + b
        return A.T.copy().astype(f), Bm.T.copy().astype(f), c0.reshape(1, -1).astype(f)

    d = {}
    d["AT0"], d["BT0"], d["c00"] = edge(inputs["conv1_w"], inputs["bn1_g"],
                                        inputs["bn1_b"], inputs["conv1_b"])
    d["AT1"], d["BT1"], d["c01"] = edge(inputs["conv2_w"], inputs["bn2_g"], inputs["bn2_b"])
    d["AT2"], d["BT2"], d["c02"] = edge(inputs["conv3_w"], inputs["bn3_g"], inputs["bn3_b"])
    d["AT3"], d["BT3"], d["c03"] = edge(inputs["conv4_w"], inputs["bn4_g"], inputs["bn4_b"])
    h = np.float16
    d["w5T"] = inputs["conv5_w"].T.copy().astype(h)
    s6 = s(inputs["bn6_g"])
    d["l1T"] = (s6[:, None] * inputs["lin1_w"]).T.copy().astype(h)
    d["b6"] = inputs["bn6_b"].reshape(1, -1).astype(h)
    s7 = s(inputs["bn7_g"])
    d["l2T"] = (s7[:, None] * inputs["lin2_w"]).T.copy().astype(h)
    d["c7"] = (s7 * inputs["lin2_b"] + inputs["bn7_b"]).reshape(1, -1).astype(h)
    d["l3T"] = inputs["lin3_w"].T.copy().astype(h)
    d["b8"] = inputs["lin3_b"].reshape(1, -1).astype(h)
    return d


def _install_ntff_hook():
    """The agent image's antenv lacks axon_hooks; synthesize it and register
    the ctypes NTFF profiling hook from trn_agent_boot (same as trn_boot)."""
    import sys
    import types

    if "antenv.axon_hooks" in sys.modules:
        return
    import antenv

    mod = types.ModuleType("antenv.axon_hooks")
    holder = [None]
    mod.set_axon_ntff_profile_hook = lambda h: holder.__setitem__(0, h)
    mod.get_axon_ntff_profile_hook = lambda: holder[0]
    sys.modules["antenv.axon_hooks"] = mod
    antenv.axon_hooks = mod
    try:
        from trn_agent_boot.trn_boot import _ntff_profile_via_ctypes

        mod.set_axon_ntff_profile_hook(
            _ntff_profile_via_ctypes("/opt/axon/libaxon_pjrt.so"))
    except Exception as e:
        print(f"NTFF hook install failed: {e}")


def kernel(**inputs):
    global LAST_RESULTS
    from concourse.bass_utils import run_bass_kernel_spmd

    if "nc" not in _CACHE:
        _CACHE["nc"] = _build()
    nc = _CACHE["nc"]

    x = np.asarray(inputs["x"], dtype=np.float32)  # (8, 1024, 3)
    common = _prep_inputs({k: np.asarray(v) for k, v in inputs.items()})
    in_maps = [dict(common, xin=np.ascontiguousarray(x[i].T)) for i in range(NCORES)]

    trace = bool(int(os.environ.get("DGCNN_TRACE", "0")))
    if trace:
        _install_ntff_hook()
    res = run_bass_kernel_spmd(nc, in_maps, core_ids=list(range(NCORES)),
                               trace=trace, trace_cores=[0] if trace else None)
    LAST_RESULTS = res
    out = np.stack([r["out"].reshape(40) for r in res.results]).astype(np.float32)
    return out



# revision 54
# speedup vs baseline: 1.5907x; 1.0346x over previous
"""DGCNN forward on 8 Trainium2 NeuronCores (Bass/Tile), pure data parallel.

Each core processes one sample (N=1024 points, K=20 neighbors).

Algorithmic mapping per EdgeConv layer (weights BN-folded on host):
  y[:,n,j] = Wd@(x_nbr - x_ctr) + Wc@x_ctr   (1x1 conv on edge features)
           = Wd@x[:,idx[n,j]] + (Wc-Wd)@x[:,n]
  After folding the (eval-mode) BN scale s and bias into the weights, and
  because max over neighbors commutes with the monotone LeakyReLU:
  out[:,n] = lrelu( max_j u[:,idx[n,j]] + v[:,n] )
  with u = (s*Wd)@x + (s*bias + b)  and  v = (s*(Wc-Wd))@x.

  KNN row scores: top-20 of  s[n,m] = <x_n,x_m> - ||x_m||^2/2  (equivalent
  ordering to the reference's -||x_n-x_m||^2 per row).

Top-20 per row: 3 rounds of DVE max8 / max_index / match_replace.
Neighbor gather: gpsimd dma_gather of u^T rows from DRAM, split over the
4 SWDGE queues; reduce-max over the 20 gathered rows on the Pool engine.
"""

import os

import numpy as np

N = 1024
K = 20
NCORES = 8
EPS = 1e-5
SLOPE = 0.01
NEG = -3.0e38

# (C_in, O) per edge conv layer
EDGE_LAYERS = [(3, 64), (64, 64), (64, 128), (128, 256)]

_CACHE = {}
LAST_RESULTS = None


def _build():
    import concourse.bass as bass
    import concourse.mybir as mybir
    import concourse.tile as tile
    from concourse import bacc

    dt = mybir.dt
    f32 = dt.float32
    u16 = dt.uint16
    f16 = dt.float16
    i16 = dt.int16
    Alu = mybir.AluOpType
    Act = mybir.ActivationFunctionType
    AX = mybir.AxisListType
    f32r = dt.float32r

    nc = bacc.Bacc("TRN2", target_bir_lowering=False, debug=False,
                   num_swdge_queues=4)
    from concourse.masks import make_identity

    def mmr(out, lhsT, rhs, **kw):
        # fp32 matmul at full PE rate via the float32r replication mode
        nc.tensor.matmul(out, lhsT.bitcast(f32r), rhs.bitcast(f32r), **kw)

    # ---------------- DRAM I/O ----------------
    xin = nc.dram_tensor("xin", [3, N], f32, kind="ExternalInput")
    ATs, BTs, c0s = [], [], []
    for li, (C, O) in enumerate(EDGE_LAYERS):
        ATs.append(nc.dram_tensor(f"AT{li}", [C, O], f32, kind="ExternalInput"))
        BTs.append(nc.dram_tensor(f"BT{li}", [C, O], f32, kind="ExternalInput"))
        c0s.append(nc.dram_tensor(f"c0{li}", [1, O], f32, kind="ExternalInput"))
    # conv5 + MLP head run in fp16 (full PE rate); weights cast on host
    w5T = nc.dram_tensor("w5T", [512, 1024], f16, kind="ExternalInput")
    l1T = nc.dram_tensor("l1T", [1024, 512], f16, kind="ExternalInput")
    b6 = nc.dram_tensor("b6", [1, 512], f16, kind="ExternalInput")
    l2T = nc.dram_tensor("l2T", [512, 256], f16, kind="ExternalInput")
    c7 = nc.dram_tensor("c7", [1, 256], f16, kind="ExternalInput")
    l3T = nc.dram_tensor("l3T", [256, 40], f16, kind="ExternalInput")
    b8 = nc.dram_tensor("b8", [1, 40], f16, kind="ExternalInput")
    out_d = nc.dram_tensor("out", [40, 1], f32, kind="ExternalOutput")

    with tile.TileContext(nc) as tc, __import__("contextlib").ExitStack() as ctx:
        const = ctx.enter_context(tc.tile_pool(name="const", bufs=1))
        xpool = ctx.enter_context(tc.tile_pool(name="xpool", bufs=1))
        work = ctx.enter_context(tc.tile_pool(name="work", bufs=2))
        srow_p = ctx.enter_context(tc.tile_pool(name="srow", bufs=3))
        gth_p = ctx.enter_context(tc.tile_pool(name="gth", bufs=20))
        vt_p = ctx.enter_context(tc.tile_pool(name="vt", bufs=3))
        small = ctx.enter_context(tc.tile_pool(name="small", bufs=4))
        mm = ctx.enter_context(tc.tile_pool(name="mm", bufs=4, space="PSUM"))
        sm = ctx.enter_context(tc.tile_pool(name="sm", bufs=3, space="PSUM"))
        tp = ctx.enter_context(tc.tile_pool(name="tp", bufs=1, space="PSUM"))
        dram = ctx.enter_context(tc.tile_pool(name="dram", bufs=2, space="DRAM"))
        dram_s = ctx.enter_context(tc.tile_pool(name="dram_s", bufs=3, space="DRAM"))

        # persistent channel-major feature tiles; layers whose KNN input has
        # C<128 carry an extra all-ones channel row used to fold the -xx/2
        # rank-1 term into the single distance matmul (lhsT side).
        # x0 is padded to 33 rows: x in rows 0:3, zeros in 3:32 (so they
        # contribute nothing to the K=33 distance matmul), ones row at 32
        # (engine writes must start at a 32-aligned partition).
        x0 = xpool.tile([33, 1024], f32, tag="x0")
        x1 = xpool.tile([65, 1024], f32, tag="x1")
        x2 = xpool.tile([65, 1024], f32, tag="x2")
        x3 = xpool.tile([128, 1024], f32, tag="x3")
        x4a = xpool.tile([128, 1024], f32, tag="x4a")
        x4b = xpool.tile([128, 1024], f32, tag="x4b")
        gp = xpool.tile([128, 8], f32, tag="gp")
        nc.vector.memset(x0[:], 0.0)
        nc.vector.memset(x0[32:33, :], 1.0)
        nc.vector.memset(x1[64:65, :], 1.0)
        nc.vector.memset(x2[64:65, :], 1.0)

        # load x (host pre-transposed channel-major [3, 1024]; a device-side
        # transposing DMA would be 4B-granule descriptor hell) FIRST so L1
        # isn't blocked behind the ~3MB of weight loads below.
        nc.sync.dma_start(x0[0:3, :], xin.ap())

        # ------------- constants into SBUF -------------
        def load_const(name, dram_t, shape=None, dtype=None):
            t = const.tile(list(shape or dram_t.shape), dtype or f32, tag=name)
            nc.sync.dma_start(t[:], dram_t.ap())
            return t

        AT_sb = [load_const(f"AT{i}", ATs[i]) for i in range(4)]
        BT_sb = [load_const(f"BT{i}", BTs[i]) for i in range(4)]
        c0_sb = [load_const(f"c0{i}", c0s[i]) for i in range(4)]
        b6_sb = load_const("b6", b6, dtype=f16)
        c7_sb = load_const("c7", c7, dtype=f16)
        b8_sb = load_const("b8", b8, dtype=f16)

        # w5T: 5 K-chunks matching [x1(64), x2(64), x3(128), x4a(128), x4b(128)]
        # fp16: conv5 + head run at full PE rate
        w5_rows = [(0, 64), (64, 128), (128, 256), (256, 384), (384, 512)]
        w5_sb = []
        for i, (r0, r1) in enumerate(w5_rows):
            t = const.tile([r1 - r0, 1024], f16, tag=f"w5_{i}")
            nc.sync.dma_start(t[:], w5T.ap()[r0:r1, :])
            w5_sb.append(t)
        l1_sb = []
        for k in range(8):
            t = const.tile([128, 512], f16, tag=f"l1_{k}")
            nc.sync.dma_start(t[:], l1T.ap()[k * 128:(k + 1) * 128, :])
            l1_sb.append(t)
        l2_sb = []
        for k in range(4):
            t = const.tile([128, 256], f16, tag=f"l2_{k}")
            nc.sync.dma_start(t[:], l2T.ap()[k * 128:(k + 1) * 128, :])
            l2_sb.append(t)
        l3_sb = []
        for k in range(2):
            t = const.tile([128, 40], f16, tag=f"l3_{k}")
            nc.sync.dma_start(t[:], l3T.ap()[k * 128:(k + 1) * 128, :])
            l3_sb.append(t)

        ones_col = const.tile([128, 1], f32, tag="ones_col")
        nc.vector.memset(ones_col[:], 1.0)
        ones_row = const.tile([1, 128], f32, tag="ones_row")
        nc.vector.memset(ones_row[:], 1.0)
        ones16 = const.tile([1, 128], f16, tag="ones16")
        nc.vector.memset(ones16[:], 1.0)
        # f16 identity for PE-transpose of the topk index tiles (indices are
        # moved as exact f16 integer values; Ldweights only takes fp dtypes)
        ident = const.tile([128, 128], f16, tag="ident")
        make_identity(nc, ident[:])

        # dummy dma_gather at startup: pulls the Pool engine's SWDGE ucode
        # LIBRARY_RELOAD (~10us) off L1's first-gather critical path
        warm_idx = const.tile([16, 8], i16, tag="warm_idx")
        nc.vector.memset(warm_idx[:], 0)
        warm_out = const.tile([128, 128], f16, tag="warm_out")
        nc.gpsimd.dma_gather(
            warm_out[:].rearrange("p (f i) -> p f i", f=1),
            w5T.ap()[:, 0:128], warm_idx[:], 128, 128, 128,
            elem_step=1024, transpose=True, queue_num=0)

        # fp16 copies of the edge-conv outputs, consumed by the fp16 conv5.
        # Split into point-halves so conv5's half-h matmuls become ready as
        # soon as the producing layer's first/last 4 chunks finish (lets the
        # scheduler overlap conv5 with L4's DVE-bound topk phase).
        def half_tiles(rows, tag):
            return [xpool.tile([rows, 512], f16, tag=f"{tag}_{h}",
                               name=f"{tag}_{h}")
                    for h in range(2)]
        xh1 = half_tiles(64, "xh1")
        xh2 = half_tiles(64, "xh2")
        xh3 = half_tiles(128, "xh3")
        xh4a = half_tiles(128, "xh4a")
        xh4b = half_tiles(128, "xh4b")


        # ---------------- edge conv layer ----------------
        def edge_layer(li, xch, C, O, xouts):
            """xch: [C, 1024] channel-major SBUF AP.
            xouts: list of [rows, 1024] channel-major target tiles, one per
            128-channel block (block f holds channels f*128..)."""
            Opad = max(O, 128)       # gather element = Opad fp16 (>=256B)
            Of = Opad // 128

            # u^T rows (fp16) to DRAM as the gather source
            uTd = dram.tile([N, 256], f16, tag="uTd")
            u_src = uTd[:, 0:Opad]   # ap[0] step 256 -> elem_step

            # xx = colsum(x*x). For C<128 build xar = [x; -xx/2] so the
            # distance block is ONE matmul (lhsT = [x; ones], rhs = xar).
            aug = C < 128
            onesrow = 32 if C < 32 else C   # 32-aligned for L1 (C=3)
            Ka = onesrow + 1
            xsq = work.tile([C, 1024], f32, tag="xsq")
            for mq in range(8):
                qsl = slice(mq * 128, (mq + 1) * 128)
                nc.vector.tensor_mul(xsq[:, qsl], xch[0:C, qsl],
                                     xch[0:C, qsl])
            if aug:
                xar = work.tile([Ka, 1024], f32, tag="xar")
                if onesrow > C:
                    # rows C..onesrow pair with zero rows of xch in the
                    # augmented matmul; they must not hold NaN garbage
                    nc.vector.memset(xar[0:onesrow, :], 0.0)
                for mq in range(8):
                    qsl = slice(mq * 128, (mq + 1) * 128)
                    nc.scalar.activation(xar[0:C, qsl], xch[0:C, qsl],
                                         Act.Copy)
                nxx = xar[onesrow:Ka, :]
            else:
                nxxt = work.tile([1, 1024], f32, tag="nxx")
                nxx = nxxt[:]
            for h in range(2):
                ps = mm.tile([1, 512], f32, tag="mm")
                nc.tensor.matmul(ps[:], ones_col[0:C, :],
                    xsq[:, h * 512:(h + 1) * 512])
                nc.scalar.activation(nxx[0:1, h * 512:(h + 1) * 512], ps[:],
                                     Act.Copy, scale=-0.5)
            # u^T per 128-point chunk (point-major, fp16, with c0 folded in)
            for m in range(8):
                csl = slice(m * 128, (m + 1) * 128)
                pu = sm.tile([128, O], f32, tag="sm")
                nc.tensor.matmul(pu[:], xch[0:C, csl], AT_sb[li][:],
                    start=True, stop=False)
                nc.tensor.matmul(pu[:], ones_row[:], c0_sb[li][:],
                    start=False, stop=True)
                uT = work.tile([128, O], f16, tag="uT", bufs=3)
                nc.scalar.activation(uT[:], pu[:], Act.Copy)
                nc.scalar.dma_start(uTd[csl, 0:O], uT[:])

            # v channel-major [O, 1024] (f32)
            vs = []
            for f in range(Of if O >= 128 else 1):
                osl = slice(f * 128, min((f + 1) * 128, O))
                orows = osl.stop - osl.start
                vt = vt_p.tile([128, 1024], f32, tag="vt")
                for h in range(2):
                    nsl = slice(h * 512, (h + 1) * 512)
                    pv = mm.tile([128, 512], f32, tag="mm")
                    nc.tensor.matmul(pv[0:orows, :], BT_sb[li][:, osl],
                        xch[0:C, nsl])
                    nc.scalar.activation(vt[0:orows, nsl], pv[0:orows, :], Act.Copy)
                vs.append(vt)

            # gather pipeline: shuffle chunk m's idx rows, then its 4
            # queue-split gathers (emitted one chunk behind the topk loop so
            # Pool desc-gen and the DMA rings overlap the next topk)
            gq_tiles = {}

            def emit_gather(m):
                idxs = idx_tiles[m]
                gqs = []
                for q in range(4):
                    gq = gth_p.tile([128, Of * 640], f16, tag="gth")
                    nc.gpsimd.dma_gather(
                        gq[:].rearrange("p (f i) -> p f i", f=Of),
                        u_src,
                        idxs[:, q * 40:(q + 1) * 40],
                        640, 640, Opad, elem_step=256, transpose=True,
                        queue_num=q,
                    )
                    gqs.append(gq)
                gq_tiles[m] = gqs

            # neighbor max for chunk m (fp16 trees on the Pool engine, which
            # is otherwise idle between gather desc-gens), then
            # x_next = lrelu(mk + v) channel-major (DVE)
            def emit_tree(m):
                csl = slice(m * 128, (m + 1) * 128)
                mkT = small.tile([128, Of * 128], f16, tag="mkT")
                for f in range(Of):
                    ga, gb, gc, gd = (
                        g[:].rearrange("p (ff i) -> p ff i", ff=Of)[:, f, :]
                        for g in gq_tiles[m])
                    # cross-queue max first at full width (fewer, larger DVE
                    # ops: instruction overhead dominates [128,128] pieces)
                    tA = small.tile([128, 640], f16, tag="tA", bufs=2)
                    nc.vector.tensor_tensor(out=tA[:], in0=ga, in1=gb,
                                            op=Alu.max)
                    tB = small.tile([128, 640], f16, tag="tB", bufs=2)
                    nc.vector.tensor_tensor(out=tB[:], in0=gc, in1=gd,
                                            op=Alu.max)
                    tC = small.tile([128, 640], f16, tag="tC", bufs=2)
                    nc.vector.tensor_tensor(out=tC[:], in0=tA[:], in1=tB[:],
                                            op=Alu.max)
                    tD = small.tile([128, 256], f16, tag="tD", bufs=2)
                    nc.vector.tensor_tensor(out=tD[:], in0=tC[:, 0:256],
                                            in1=tC[:, 256:512], op=Alu.max)
                    tE = small.tile([128, 128], f16, tag="tE", bufs=2)
                    nc.vector.tensor_tensor(out=tE[:], in0=tD[:, 0:128],
                                            in1=tD[:, 128:256], op=Alu.max)
                    nc.vector.tensor_tensor(
                        out=mkT[:, f * 128:(f + 1) * 128], in0=tE[:],
                        in1=tC[:, 512:640], op=Alu.max)

                for f, (xt, xh, rows) in enumerate(xouts):
                    z = small.tile([128, 128], f32, tag="z")
                    nc.vector.tensor_add(z[0:rows, :],
                                         mkT[0:rows, f * 128:f * 128 + 128],
                                         vs[f][0:rows, csl])
                    nc.vector.scalar_tensor_tensor(
                        out=xt[0:rows, csl], in0=z[0:rows, :], scalar=SLOPE,
                        in1=z[0:rows, :], op0=Alu.mult, op1=Alu.max)
                    # fp16 copy for conv5 (ACT engine, off the DVE path)
                    lsl = slice((m % 4) * 128, (m % 4) * 128 + 128)
                    nc.scalar.activation(xh[m // 4][0:rows, lsl],
                                         xt[0:rows, csl], Act.Copy)

            # ---- phase B: dist + topk + idx wrap, all 8 chunks, with the
            # previous chunk's gather and the chunk-before-that's tree
            # emitted one/two behind so every engine pipelines ----
            idx_tiles = []
            for m in range(8):
                csl = slice(m * 128, (m + 1) * 128)
                srow = srow_p.tile([128, 1024], f32, tag="srow")
                for h in range(2):
                    nsl = slice(h * 512, (h + 1) * 512)
                    pd = mm.tile([128, 512], f32, tag="mm")
                    if aug:
                        nc.tensor.matmul(pd[:], xch[0:Ka, csl],
                            xar[:, nsl])
                    else:
                        nc.tensor.matmul(pd[:], xch[0:C, csl], xch[0:C, nsl],
                            start=True, stop=False)
                        nc.tensor.matmul(pd[:], ones_row[:], nxx[0:1, nsl],
                            start=False, stop=True)
                    nc.scalar.activation(srow[:, nsl], pd[:], Act.Copy)

                idx20 = small.tile([128, 20], u16, tag="idx20", bufs=6)
                idx8b = small.tile([128, 8], u16, tag="idx8b")
                for r in range(3):
                    m8 = small.tile([128, 8], f32, tag="m8")
                    nc.vector.max(m8[:], srow[:])
                    dst8 = idx20[:, r * 8:r * 8 + 8] if r < 2 else idx8b[:]
                    nc.vector.max_index(dst8, m8[:], srow[:])
                    if r < 2:
                        nc.vector.match_replace(srow[:], m8[:], srow[:], NEG)
                nc.vector.tensor_copy(idx20[:, 16:20], idx8b[:, 0:4])

                # Gather flat order is c-major: flat[i] = idx[point i%128]
                # [nbr i//128]; wrapped layout sbuf[q, s] = flat[s*16+q] per
                # 32-partition quadrant. Affine in (t=p//16, q=p%16, c):
                # F2W[q*160 + c*8 + t] = idx20[t*16+q, c].
                # Build it via PE transpose (u16 pass-through) + a free-dim
                # permute copy, so the DRAM write is 16B-contiguous runs
                # instead of scattered 2B granules.
                idxf = work.tile([128, 20], f16, tag="idxf", bufs=4)
                nc.vector.tensor_copy(idxf[:], idx20[:])
                tpp = tp.tile([20, 128], f16, tag="tp")
                # high priority: the transpose gates the whole gather path
                # and must not queue behind the run-ahead dist matmuls on PE
                with tc.high_priority():
                    nc.tensor.transpose(tpp[:], idxf[:], ident[:])
                tcw = work.tile([20, 128], u16, tag="tcw", bufs=4)
                nc.vector.tensor_copy(
                    tcw[:].rearrange("c (q t) -> c t q", q=16, t=8),
                    tpp[:])
                F2W = dram_s.tile([2560], u16, tag="F2")
                w4 = F2W[:].rearrange("(q c t) -> c q t", q=16, c=20, t=8)
                nc.sync.dma_start(w4, tcw[:])
                # read back replicated into all 8 16-partition groups (the
                # layout dma_gather's SWDGE descriptor generator expects)
                idxw = work.tile([128, 160], i16, tag="idxw", bufs=9)
                rdq = F2W[:].bitcast(i16).rearrange("(q s) -> q s", q=16)
                for r in range(8):
                    nc.sync.dma_start(idxw[16 * r:16 * r + 16, :], rdq)
                idx_tiles.append(idxw)
                emit_gather(m)
                lag = 5
                if m >= lag:
                    emit_tree(m - lag)
            for mm_ in range(8 - lag, 8):
                emit_tree(mm_)

        edge_layer(0, x0[:], 3, 64, [(x1, xh1, 64)])
        edge_layer(1, x1[:], 64, 64, [(x2, xh2, 64)])
        edge_layer(2, x2[:], 64, 128, [(x3, xh3, 128)])
        edge_layer(3, x3[:], 128, 256, [(x4a, xh4a, 128), (x4b, xh4b, 128)])

        # ---------------- conv5 (512 -> 1024) + global max pool ----------------
        # conv5 PSUM banks are drained by ACT (fp16 evac) so the PE can pack
        # conv5 groups into L4's idle stretches without waiting on the busy
        # DVE; the global max pool then runs as 8 cheap rowmax ops at the end.
        xc_full = [(xh1, 64), (xh2, 64), (xh3, 128), (xh4a, 128), (xh4b, 128)]
        e_sb = []
        for mo in range(8):
            msl = slice(mo * 128, (mo + 1) * 128)
            et = xpool.tile([128, 1024], f16, tag=f"e{mo}", name=f"e{mo}")
            for h in range(2):
                pe = mm.tile([128, 512], f32, tag="mm")
                for k in range(5):
                    nc.tensor.matmul(pe[:], w5_sb[k][:, msl],
                        xc_full[k][0][h][0:xc_full[k][1], :],
                        start=(k == 0), stop=(k == 4))
                nc.scalar.activation(et[:, h * 512:(h + 1) * 512], pe[:],
                                     Act.Copy)
            e_sb.append(et)
        for mo in range(8):
            nc.vector.reduce_max(gp[:, mo:mo + 1], e_sb[mo][:], axis=AX.X)

        # ---------------- MLP head (fp16 weights/activations) ----------------
        gph = small.tile([128, 8], f16, tag="gph")
        nc.scalar.activation(gph[:], gp[:], Act.Copy)
        y1 = small.tile([128, 4], f16, tag="y1")
        for mt in range(4):
            msl = slice(mt * 128, (mt + 1) * 128)
            p1 = sm.tile([128, 1], f32, tag="sm")
            for k in range(8):
                nc.tensor.matmul(p1[:], l1_sb[k][:, msl], gph[:, k:k + 1],
                    start=(k == 0), stop=False)
            nc.tensor.matmul(p1[:], b6_sb[0:1, msl], ones16[0:1, 0:1],
                start=False, stop=True)
            t1 = small.tile([128, 1], f32, tag="t1")
            nc.scalar.activation(t1[:], p1[:], Act.Copy)
            nc.vector.scalar_tensor_tensor(
                out=y1[:, mt:mt + 1], in0=t1[:], scalar=SLOPE, in1=t1[:],
                op0=Alu.mult, op1=Alu.max)

        y2 = small.tile([128, 2], f16, tag="y2")
        for mt in range(2):
            msl = slice(mt * 128, (mt + 1) * 128)
            p2 = sm.tile([128, 1], f32, tag="sm")
            for k in range(4):
                nc.tensor.matmul(p2[:], l2_sb[k][:, msl], y1[:, k:k + 1],
                    start=(k == 0), stop=False)
            nc.tensor.matmul(p2[:], c7_sb[0:1, msl], ones16[0:1, 0:1],
                start=False, stop=True)
            t2 = small.tile([128, 1], f32, tag="t2")
            nc.scalar.activation(t2[:], p2[:], Act.Copy)
            nc.vector.scalar_tensor_tensor(
                out=y2[:, mt:mt + 1], in0=t2[:], scalar=SLOPE, in1=t2[:],
                op0=Alu.mult, op1=Alu.max)

        p3 = sm.tile([40, 1], f32, tag="sm")
        for k in range(2):
            nc.tensor.matmul(p3[:], l3_sb[k][:, 0:40], y2[:, k:k + 1],
                start=(k == 0), stop=False)
        nc.tensor.matmul(p3[:], b8_sb[0:1, 0:40], ones16[0:1, 0:1],
            start=False, stop=True)
        y3 = small.tile([40, 1], f32, tag="y3")
        nc.scalar.activation(y3[:], p3[:], Act.Copy)
        nc.sync.dma_start(out_d.ap(), y3[:])

    nc.compile()
    return nc


def _prep_inputs(inputs):
    """Fold eval-mode BN into conv/linear weights; transpose for the device."""
    f = np.float32
    s = lambda g: (g / np.sqrt(f(1.0) + f(EPS))).astype(f)

    def edge(w, g, b, bias=None):
        O, C2 = w.shape
        C = C2 // 2
        sc = s(g)
        Wd = w[:, :C]
        Wc = w[:, C:]
        A = sc[:, None] * Wd
        Bm = sc[:, None] * (Wc - Wd)
        c0 = sc * (bias if bias is not None else 0.0) + b
        return A.T.copy().astype(f), Bm.T.copy().astype(f), c0.reshape(1, -1).astype(f)

    d = {}
    d["AT0"], d["BT0"], d["c00"] = edge(inputs["conv1_w"], inputs["bn1_g"],
                                        inputs["bn1_b"], inputs["conv1_b"])
    d["AT1"], d["BT1"], d["c01"] = edge(inputs["conv2_w"], inputs["bn2_g"], inputs["bn2_b"])
    d["AT2"], d["BT2"], d["c02"] = edge(inputs["conv3_w"], inputs["bn3_g"], inputs["bn3_b"])
    d["AT3"], d["BT3"], d["c03"] = edge(inputs["conv4_w"], inputs["bn4_g"], inputs["bn4_b"])
    h = np.float16
    d["w5T"] = inputs["conv5_w"].T.copy().astype(h)
    s6 = s(inputs["bn6_g"])
    d["l1T"] = (s6[:, None] * inputs["lin1_w"]).T.copy().astype(h)
    d["b6"] = inputs["bn6_b"].reshape(1, -1).astype(h)
    s7 = s(inputs["bn7_g"])
    d["l2T"] = (s7[:, None] * inputs["lin2_w"]).T.copy().astype(h)
    d["c7"] = (s7 * inputs["lin2_b"] + inputs["bn7_b"]).reshape(1, -1).astype(h)
    d["l3T"] = inputs["lin3_w"].T.copy().astype(h)
    d["b8"] = inputs["lin3_b"].reshape(1, -1).astype(h)
    return d


def _install_ntff_hook():
    """The agent image's antenv lacks axon_hooks; synthesize it and register
    the ctypes NTFF profiling hook from trn_agent_boot (same as trn_boot)."""
    import sys
    import types

    if "antenv.axon_hooks" in sys.modules:
        return
    import antenv

    mod = types.ModuleType("antenv.axon_hooks")
    holder = [None]
    mod.set_axon_ntff_profile_hook = lambda h: holder.__setitem__(0, h)
    mod.get_axon_ntff_profile_hook = lambda: holder[0]
    sys.modules["antenv.axon_hooks"] = mod
    antenv.axon_hooks = mod
    try:
        from trn_agent_boot.trn_boot import _ntff_profile_via_ctypes

        mod.set_axon_ntff_profile_hook(
            _ntff_profile_via_ctypes("/opt/axon/libaxon_pjrt.so"))
    except Exception as e:
        print(f"NTFF hook install failed: {e}")


def kernel(**inputs):
    global LAST_RESULTS
    from concourse.bass_utils import run_bass_kernel_spmd

    if "nc" not in _CACHE:
        _CACHE["nc"] = _build()
    nc = _CACHE["nc"]

    x = np.asarray(inputs["x"], dtype=np.float32)  # (8, 1024, 3)
    common = _prep_inputs({k: np.asarray(v) for k, v in inputs.items()})
    in_maps = [dict(common, xin=np.ascontiguousarray(x[i].T)) for i in range(NCORES)]

    trace = bool(int(os.environ.get("DGCNN_TRACE", "0")))
    if trace:
        _install_ntff_hook()
    res = run_bass_kernel_spmd(nc, in_maps, core_ids=list(range(NCORES)),
                               trace=trace, trace_cores=[0] if trace else None)
    LAST_RESULTS = res
    out = np.stack([r["out"].reshape(40) for r in res.results]).astype(np.float32)
    return out



# revision 55
# speedup vs baseline: 1.6055x; 1.0093x over previous
"""DGCNN forward on 8 Trainium2 NeuronCores (Bass/Tile), pure data parallel.

Each core processes one sample (N=1024 points, K=20 neighbors).

Algorithmic mapping per EdgeConv layer (weights BN-folded on host):
  y[:,n,j] = Wd@(x_nbr - x_ctr) + Wc@x_ctr   (1x1 conv on edge features)
           = Wd@x[:,idx[n,j]] + (Wc-Wd)@x[:,n]
  After folding the (eval-mode) BN scale s and bias into the weights, and
  because max over neighbors commutes with the monotone LeakyReLU:
  out[:,n] = lrelu( max_j u[:,idx[n,j]] + v[:,n] )
  with u = (s*Wd)@x + (s*bias + b)  and  v = (s*(Wc-Wd))@x.

  KNN row scores: top-20 of  s[n,m] = <x_n,x_m> - ||x_m||^2/2  (equivalent
  ordering to the reference's -||x_n-x_m||^2 per row).

Top-20 per row: 3 rounds of DVE max8 / max_index / match_replace.
Neighbor gather: gpsimd dma_gather of u^T rows from DRAM, split over the
4 SWDGE queues; reduce-max over the 20 gathered rows on the Pool engine.
"""

import os

import numpy as np

N = 1024
K = 20
NCORES = 8
EPS = 1e-5
SLOPE = 0.01
NEG = -3.0e38

# (C_in, O) per edge conv layer
EDGE_LAYERS = [(3, 64), (64, 64), (64, 128), (128, 256)]

_CACHE = {}
LAST_RESULTS = None


def _build():
    import concourse.bass as bass
    import concourse.mybir as mybir
    import concourse.tile as tile
    from concourse import bacc

    dt = mybir.dt
    f32 = dt.float32
    u16 = dt.uint16
    f16 = dt.float16
    i16 = dt.int16
    Alu = mybir.AluOpType
    Act = mybir.ActivationFunctionType
    AX = mybir.AxisListType
    f32r = dt.float32r

    nc = bacc.Bacc("TRN2", target_bir_lowering=False, debug=False,
                   num_swdge_queues=4)
    from concourse.masks import make_identity

    def mmr(out, lhsT, rhs, **kw):
        # fp32 matmul at full PE rate via the float32r replication mode
        nc.tensor.matmul(out, lhsT.bitcast(f32r), rhs.bitcast(f32r), **kw)

    # ---------------- DRAM I/O ----------------
    xin = nc.dram_tensor("xin", [3, N], f32, kind="ExternalInput")
    ATs, BTs, c0s = [], [], []
    for li, (C, O) in enumerate(EDGE_LAYERS):
        ATs.append(nc.dram_tensor(f"AT{li}", [C, O], f32, kind="ExternalInput"))
        BTs.append(nc.dram_tensor(f"BT{li}", [C, O], f32, kind="ExternalInput"))
        c0s.append(nc.dram_tensor(f"c0{li}", [1, O], f32, kind="ExternalInput"))
    # conv5 + MLP head run in fp16 (full PE rate); weights cast on host
    w5T = nc.dram_tensor("w5T", [512, 1024], f16, kind="ExternalInput")
    l1T = nc.dram_tensor("l1T", [1024, 512], f16, kind="ExternalInput")
    b6 = nc.dram_tensor("b6", [1, 512], f16, kind="ExternalInput")
    l2T = nc.dram_tensor("l2T", [512, 256], f16, kind="ExternalInput")
    c7 = nc.dram_tensor("c7", [1, 256], f16, kind="ExternalInput")
    l3T = nc.dram_tensor("l3T", [256, 40], f16, kind="ExternalInput")
    b8 = nc.dram_tensor("b8", [1, 40], f16, kind="ExternalInput")
    out_d = nc.dram_tensor("out", [40, 1], f32, kind="ExternalOutput")

    with tile.TileContext(nc) as tc, __import__("contextlib").ExitStack() as ctx:
        const = ctx.enter_context(tc.tile_pool(name="const", bufs=1))
        xpool = ctx.enter_context(tc.tile_pool(name="xpool", bufs=1))
        work = ctx.enter_context(tc.tile_pool(name="work", bufs=2))
        srow_p = ctx.enter_context(tc.tile_pool(name="srow", bufs=2))
        gth_p = ctx.enter_context(tc.tile_pool(name="gth", bufs=20))
        vt_p = ctx.enter_context(tc.tile_pool(name="vt", bufs=3))
        small = ctx.enter_context(tc.tile_pool(name="small", bufs=4))
        mm = ctx.enter_context(tc.tile_pool(name="mm", bufs=4, space="PSUM"))
        sm = ctx.enter_context(tc.tile_pool(name="sm", bufs=3, space="PSUM"))
        tp = ctx.enter_context(tc.tile_pool(name="tp", bufs=1, space="PSUM"))
        dram = ctx.enter_context(tc.tile_pool(name="dram", bufs=2, space="DRAM"))
        dram_s = ctx.enter_context(tc.tile_pool(name="dram_s", bufs=3, space="DRAM"))

        # persistent channel-major feature tiles; layers whose KNN input has
        # C<128 carry an extra all-ones channel row used to fold the -xx/2
        # rank-1 term into the single distance matmul (lhsT side).
        # x0 is padded to 33 rows: x in rows 0:3, zeros in 3:32 (so they
        # contribute nothing to the K=33 distance matmul), ones row at 32
        # (engine writes must start at a 32-aligned partition).
        x0 = xpool.tile([33, 1024], f32, tag="x0")
        x1 = xpool.tile([65, 1024], f32, tag="x1")
        x2 = xpool.tile([65, 1024], f32, tag="x2")
        x3 = xpool.tile([128, 1024], f32, tag="x3")
        x4a = xpool.tile([128, 1024], f32, tag="x4a")
        x4b = xpool.tile([128, 1024], f32, tag="x4b")
        gp = xpool.tile([128, 8], f32, tag="gp")
        nc.vector.memset(x0[:], 0.0)
        nc.vector.memset(x0[32:33, :], 1.0)
        nc.vector.memset(x1[64:65, :], 1.0)
        nc.vector.memset(x2[64:65, :], 1.0)

        # load x (host pre-transposed channel-major [3, 1024]; a device-side
        # transposing DMA would be 4B-granule descriptor hell) FIRST so L1
        # isn't blocked behind the ~3MB of weight loads below.
        nc.sync.dma_start(x0[0:3, :], xin.ap())

        # ------------- constants into SBUF -------------
        def load_const(name, dram_t, shape=None, dtype=None):
            t = const.tile(list(shape or dram_t.shape), dtype or f32, tag=name)
            nc.sync.dma_start(t[:], dram_t.ap())
            return t

        AT_sb = [load_const(f"AT{i}", ATs[i]) for i in range(4)]
        BT_sb = [load_const(f"BT{i}", BTs[i]) for i in range(4)]
        c0_sb = [load_const(f"c0{i}", c0s[i]) for i in range(4)]
        b6_sb = load_const("b6", b6, dtype=f16)
        c7_sb = load_const("c7", c7, dtype=f16)
        b8_sb = load_const("b8", b8, dtype=f16)

        # w5T: 5 K-chunks matching [x1(64), x2(64), x3(128), x4a(128), x4b(128)]
        # fp16: conv5 + head run at full PE rate
        w5_rows = [(0, 64), (64, 128), (128, 256), (256, 384), (384, 512)]
        w5_sb = []
        for i, (r0, r1) in enumerate(w5_rows):
            t = const.tile([r1 - r0, 1024], f16, tag=f"w5_{i}")
            nc.sync.dma_start(t[:], w5T.ap()[r0:r1, :])
            w5_sb.append(t)
        l1_sb = []
        for k in range(8):
            t = const.tile([128, 512], f16, tag=f"l1_{k}")
            nc.sync.dma_start(t[:], l1T.ap()[k * 128:(k + 1) * 128, :])
            l1_sb.append(t)
        l2_sb = []
        for k in range(4):
            t = const.tile([128, 256], f16, tag=f"l2_{k}")
            nc.sync.dma_start(t[:], l2T.ap()[k * 128:(k + 1) * 128, :])
            l2_sb.append(t)
        l3_sb = []
        for k in range(2):
            t = const.tile([128, 40], f16, tag=f"l3_{k}")
            nc.sync.dma_start(t[:], l3T.ap()[k * 128:(k + 1) * 128, :])
            l3_sb.append(t)

        ones_col = const.tile([128, 1], f32, tag="ones_col")
        nc.vector.memset(ones_col[:], 1.0)
        ones_row = const.tile([1, 128], f32, tag="ones_row")
        nc.vector.memset(ones_row[:], 1.0)
        ones16 = const.tile([1, 128], f16, tag="ones16")
        nc.vector.memset(ones16[:], 1.0)
        # f16 identity for PE-transpose of the topk index tiles (indices are
        # moved as exact f16 integer values; Ldweights only takes fp dtypes)
        ident = const.tile([128, 128], f16, tag="ident")
        make_identity(nc, ident[:])

        # dummy dma_gather at startup: pulls the Pool engine's SWDGE ucode
        # LIBRARY_RELOAD (~10us) off L1's first-gather critical path
        warm_idx = const.tile([16, 8], i16, tag="warm_idx")
        nc.vector.memset(warm_idx[:], 0)
        warm_out = const.tile([128, 128], f16, tag="warm_out")
        nc.gpsimd.dma_gather(
            warm_out[:].rearrange("p (f i) -> p f i", f=1),
            w5T.ap()[:, 0:128], warm_idx[:], 128, 128, 128,
            elem_step=1024, transpose=True, queue_num=0)

        # fp16 copies of the edge-conv outputs, consumed by the fp16 conv5.
        # Split into point-halves so conv5's half-h matmuls become ready as
        # soon as the producing layer's first/last 4 chunks finish (lets the
        # scheduler overlap conv5 with L4's DVE-bound topk phase).
        def half_tiles(rows, tag):
            return [xpool.tile([rows, 512], f16, tag=f"{tag}_{h}",
                               name=f"{tag}_{h}")
                    for h in range(2)]
        xh1 = half_tiles(64, "xh1")
        xh2 = half_tiles(64, "xh2")
        xh3 = half_tiles(128, "xh3")
        xh4a = half_tiles(128, "xh4a")
        xh4b = half_tiles(128, "xh4b")


        # ---------------- edge conv layer ----------------
        def edge_layer(li, xch, C, O, xouts):
            """xch: [C, 1024] channel-major SBUF AP.
            xouts: list of [rows, 1024] channel-major target tiles, one per
            128-channel block (block f holds channels f*128..)."""
            Opad = max(O, 128)       # gather element = Opad fp16 (>=256B)
            Of = Opad // 128

            # u^T rows (fp16) to DRAM as the gather source
            uTd = dram.tile([N, 256], f16, tag="uTd")
            u_src = uTd[:, 0:Opad]   # ap[0] step 256 -> elem_step

            # xx = colsum(x*x). For C<128 build xar = [x; -xx/2] so the
            # distance block is ONE matmul (lhsT = [x; ones], rhs = xar).
            aug = C < 128
            onesrow = 32 if C < 32 else C   # 32-aligned for L1 (C=3)
            Ka = onesrow + 1
            xsq = work.tile([C, 1024], f32, tag="xsq")
            for mq in range(8):
                qsl = slice(mq * 128, (mq + 1) * 128)
                nc.vector.tensor_mul(xsq[:, qsl], xch[0:C, qsl],
                                     xch[0:C, qsl])
            if aug:
                xar = work.tile([Ka, 1024], f32, tag="xar")
                if onesrow > C:
                    # rows C..onesrow pair with zero rows of xch in the
                    # augmented matmul; they must not hold NaN garbage
                    nc.vector.memset(xar[0:onesrow, :], 0.0)
                for mq in range(8):
                    qsl = slice(mq * 128, (mq + 1) * 128)
                    nc.scalar.activation(xar[0:C, qsl], xch[0:C, qsl],
                                         Act.Copy)
                nxx = xar[onesrow:Ka, :]
            else:
                nxxt = work.tile([1, 1024], f32, tag="nxx")
                nxx = nxxt[:]
            for h in range(2):
                ps = mm.tile([1, 512], f32, tag="mm")
                nc.tensor.matmul(ps[:], ones_col[0:C, :],
                    xsq[:, h * 512:(h + 1) * 512])
                nc.scalar.activation(nxx[0:1, h * 512:(h + 1) * 512], ps[:],
                                     Act.Copy, scale=-0.5)
            # u^T per 128-point chunk (point-major, fp16, with c0 folded in)
            for m in range(8):
                csl = slice(m * 128, (m + 1) * 128)
                pu = sm.tile([128, O], f32, tag="sm")
                nc.tensor.matmul(pu[:], xch[0:C, csl], AT_sb[li][:],
                    start=True, stop=False)
                nc.tensor.matmul(pu[:], ones_row[:], c0_sb[li][:],
                    start=False, stop=True)
                uT = work.tile([128, O], f16, tag="uT", bufs=3)
                nc.scalar.activation(uT[:], pu[:], Act.Copy)
                nc.scalar.dma_start(uTd[csl, 0:O], uT[:])

            # v channel-major [O, 1024] (f32)
            vs = []
            for f in range(Of if O >= 128 else 1):
                osl = slice(f * 128, min((f + 1) * 128, O))
                orows = osl.stop - osl.start
                vt = vt_p.tile([128, 1024], f32, tag="vt")
                for h in range(2):
                    nsl = slice(h * 512, (h + 1) * 512)
                    pv = mm.tile([128, 512], f32, tag="mm")
                    nc.tensor.matmul(pv[0:orows, :], BT_sb[li][:, osl],
                        xch[0:C, nsl])
                    nc.scalar.activation(vt[0:orows, nsl], pv[0:orows, :], Act.Copy)
                vs.append(vt)

            # gather pipeline: shuffle chunk m's idx rows, then its 4
            # queue-split gathers (emitted one chunk behind the topk loop so
            # Pool desc-gen and the DMA rings overlap the next topk)
            gq_tiles = {}

            def emit_gather(m):
                idxs = idx_tiles[m]
                gqs = []
                for q in range(4):
                    gq = gth_p.tile([128, Of * 640], f16, tag="gth")
                    nc.gpsimd.dma_gather(
                        gq[:].rearrange("p (f i) -> p f i", f=Of),
                        u_src,
                        idxs[:, q * 40:(q + 1) * 40],
                        640, 640, Opad, elem_step=256, transpose=True,
                        queue_num=q,
                    )
                    gqs.append(gq)
                gq_tiles[m] = gqs

            # neighbor max for chunk m (fp16 trees on the Pool engine, which
            # is otherwise idle between gather desc-gens), then
            # x_next = lrelu(mk + v) channel-major (DVE)
            def emit_tree(m):
                csl = slice(m * 128, (m + 1) * 128)
                mkT = small.tile([128, Of * 128], f16, tag="mkT")
                for f in range(Of):
                    ga, gb, gc, gd = (
                        g[:].rearrange("p (ff i) -> p ff i", ff=Of)[:, f, :]
                        for g in gq_tiles[m])
                    # cross-queue max first at full width (fewer, larger DVE
                    # ops: instruction overhead dominates [128,128] pieces)
                    tA = small.tile([128, 640], f16, tag="tA", bufs=2)
                    nc.vector.tensor_tensor(out=tA[:], in0=ga, in1=gb,
                                            op=Alu.max)
                    tB = small.tile([128, 640], f16, tag="tB", bufs=2)
                    nc.vector.tensor_tensor(out=tB[:], in0=gc, in1=gd,
                                            op=Alu.max)
                    tC = small.tile([128, 640], f16, tag="tC", bufs=2)
                    nc.vector.tensor_tensor(out=tC[:], in0=tA[:], in1=tB[:],
                                            op=Alu.max)
                    tD = small.tile([128, 256], f16, tag="tD", bufs=2)
                    nc.vector.tensor_tensor(out=tD[:], in0=tC[:, 0:256],
                                            in1=tC[:, 256:512], op=Alu.max)
                    tE = small.tile([128, 128], f16, tag="tE", bufs=2)
                    nc.vector.tensor_tensor(out=tE[:], in0=tD[:, 0:128],
                                            in1=tD[:, 128:256], op=Alu.max)
                    nc.vector.tensor_tensor(
                        out=mkT[:, f * 128:(f + 1) * 128], in0=tE[:],
                        in1=tC[:, 512:640], op=Alu.max)

                for f, (xt, xh, rows) in enumerate(xouts):
                    z = small.tile([128, 128], f32, tag="z")
                    nc.vector.tensor_add(z[0:rows, :],
                                         mkT[0:rows, f * 128:f * 128 + 128],
                                         vs[f][0:rows, csl])
                    nc.vector.scalar_tensor_tensor(
                        out=xt[0:rows, csl], in0=z[0:rows, :], scalar=SLOPE,
                        in1=z[0:rows, :], op0=Alu.mult, op1=Alu.max)
                    # fp16 copy for conv5 (ACT engine, off the DVE path)
                    lsl = slice((m % 4) * 128, (m % 4) * 128 + 128)
                    nc.scalar.activation(xh[m // 4][0:rows, lsl],
                                         xt[0:rows, csl], Act.Copy)

            # ---- phase B: dist + topk + idx wrap, all 8 chunks, with the
            # previous chunk's gather and the chunk-before-that's tree
            # emitted one/two behind so every engine pipelines ----
            idx_tiles = []
            for m in range(8):
                csl = slice(m * 128, (m + 1) * 128)
                srow = srow_p.tile([128, 1024], f32, tag="srow")
                for h in range(2):
                    nsl = slice(h * 512, (h + 1) * 512)
                    pd = mm.tile([128, 512], f32, tag="mm")
                    if aug:
                        nc.tensor.matmul(pd[:], xch[0:Ka, csl],
                            xar[:, nsl])
                    else:
                        nc.tensor.matmul(pd[:], xch[0:C, csl], xch[0:C, nsl],
                            start=True, stop=False)
                        nc.tensor.matmul(pd[:], ones_row[:], nxx[0:1, nsl],
                            start=False, stop=True)
                    nc.scalar.activation(srow[:, nsl], pd[:], Act.Copy)

                idx20 = small.tile([128, 20], u16, tag="idx20", bufs=6)
                idx8b = small.tile([128, 8], u16, tag="idx8b")
                for r in range(3):
                    m8 = small.tile([128, 8], f32, tag="m8")
                    nc.vector.max(m8[:], srow[:])
                    dst8 = idx20[:, r * 8:r * 8 + 8] if r < 2 else idx8b[:]
                    nc.vector.max_index(dst8, m8[:], srow[:])
                    if r < 2:
                        nc.vector.match_replace(srow[:], m8[:], srow[:], NEG)
                nc.vector.tensor_copy(idx20[:, 16:20], idx8b[:, 0:4])

                # Gather flat order is c-major: flat[i] = idx[point i%128]
                # [nbr i//128]; wrapped layout sbuf[q, s] = flat[s*16+q] per
                # 32-partition quadrant. Affine in (t=p//16, q=p%16, c):
                # F2W[q*160 + c*8 + t] = idx20[t*16+q, c].
                # Build it via PE transpose (u16 pass-through) + a free-dim
                # permute copy, so the DRAM write is 16B-contiguous runs
                # instead of scattered 2B granules.
                idxf = work.tile([128, 20], f16, tag="idxf", bufs=4)
                nc.vector.tensor_copy(idxf[:], idx20[:])
                tpp = tp.tile([20, 128], f16, tag="tp")
                # high priority: the transpose gates the whole gather path
                # and must not queue behind the run-ahead dist matmuls on PE
                with tc.high_priority():
                    nc.tensor.transpose(tpp[:], idxf[:], ident[:])
                tcw = work.tile([20, 128], u16, tag="tcw", bufs=4)
                nc.vector.tensor_copy(
                    tcw[:].rearrange("c (q t) -> c t q", q=16, t=8),
                    tpp[:])
                F2W = dram_s.tile([2560], u16, tag="F2")
                w4 = F2W[:].rearrange("(q c t) -> c q t", q=16, c=20, t=8)
                nc.sync.dma_start(w4, tcw[:])
                # read back replicated into all 8 16-partition groups (the
                # layout dma_gather's SWDGE descriptor generator expects)
                idxw = work.tile([128, 160], i16, tag="idxw", bufs=9)
                rdq = F2W[:].bitcast(i16).rearrange("(q s) -> q s", q=16)
                for r in range(8):
                    nc.sync.dma_start(idxw[16 * r:16 * r + 16, :], rdq)
                idx_tiles.append(idxw)
                emit_gather(m)
                lag = 5
                if m >= lag:
                    emit_tree(m - lag)
            for mm_ in range(8 - lag, 8):
                emit_tree(mm_)

        edge_layer(0, x0[:], 3, 64, [(x1, xh1, 64)])
        edge_layer(1, x1[:], 64, 64, [(x2, xh2, 64)])
        edge_layer(2, x2[:], 64, 128, [(x3, xh3, 128)])
        edge_layer(3, x3[:], 128, 256, [(x4a, xh4a, 128), (x4b, xh4b, 128)])

        # ---------------- conv5 (512 -> 1024) + global max pool ----------------
        # conv5 PSUM banks are drained by ACT (fp16 evac) so the PE can pack
        # conv5 groups into L4's idle stretches without waiting on the busy
        # DVE; the global max pool then runs as 8 cheap rowmax ops at the end.
        xc_full = [(xh1, 64), (xh2, 64), (xh3, 128), (xh4a, 128), (xh4b, 128)]
        e_sb = []
        for mo in range(8):
            msl = slice(mo * 128, (mo + 1) * 128)
            et = xpool.tile([128, 1024], f16, tag=f"e{mo}", name=f"e{mo}")
            for h in range(2):
                pe = mm.tile([128, 512], f32, tag="mm")
                for k in range(5):
                    nc.tensor.matmul(pe[:], w5_sb[k][:, msl],
                        xc_full[k][0][h][0:xc_full[k][1], :],
                        start=(k == 0), stop=(k == 4))
                nc.scalar.activation(et[:, h * 512:(h + 1) * 512], pe[:],
                                     Act.Copy)
            e_sb.append(et)
        for mo in range(8):
            nc.vector.reduce_max(gp[:, mo:mo + 1], e_sb[mo][:], axis=AX.X)

        # ---------------- MLP head (fp16 weights/activations) ----------------
        gph = small.tile([128, 8], f16, tag="gph")
        nc.scalar.activation(gph[:], gp[:], Act.Copy)
        y1 = small.tile([128, 4], f16, tag="y1")
        for mt in range(4):
            msl = slice(mt * 128, (mt + 1) * 128)
            p1 = sm.tile([128, 1], f32, tag="sm")
            for k in range(8):
                nc.tensor.matmul(p1[:], l1_sb[k][:, msl], gph[:, k:k + 1],
                    start=(k == 0), stop=False)
            nc.tensor.matmul(p1[:], b6_sb[0:1, msl], ones16[0:1, 0:1],
                start=False, stop=True)
            t1 = small.tile([128, 1], f32, tag="t1")
            nc.scalar.activation(t1[:], p1[:], Act.Copy)
            nc.vector.scalar_tensor_tensor(
                out=y1[:, mt:mt + 1], in0=t1[:], scalar=SLOPE, in1=t1[:],
                op0=Alu.mult, op1=Alu.max)

        y2 = small.tile([128, 2], f16, tag="y2")
        for mt in range(2):
            msl = slice(mt * 128, (mt + 1) * 128)
            p2 = sm.tile([128, 1], f32, tag="sm")
            for k in range(4):
                nc.tensor.matmul(p2[:], l2_sb[k][:, msl], y1[:, k:k + 1],
                    start=(k == 0), stop=False)
            nc.tensor.matmul(p2[:], c7_sb[0:1, msl], ones16[0:1, 0:1],
                start=False, stop=True)
            t2 = small.tile([128, 1], f32, tag="t2")
            nc.scalar.activation(t2[:], p2[:], Act.Copy)
            nc.vector.scalar_tensor_tensor(
                out=y2[:, mt:mt + 1], in0=t2[:], scalar=SLOPE, in1=t2[:],
                op0=Alu.mult, op1=Alu.max)

        p3 = sm.tile([40, 1], f32, tag="sm")
        for k in range(2):
            nc.tensor.matmul(p3[:], l3_sb[k][:, 0:40], y2[:, k:k + 1],
                start=(k == 0), stop=False)
        nc.tensor.matmul(p3[:], b8_sb[0:1, 0:40], ones16[0:1, 0:1],
            start=False, stop=True)
        y3 = small.tile([40, 1], f32, tag="y3")
        nc.scalar.activation(y3[:], p3[:], Act.Copy)
        nc.sync.dma_start(out_d.ap(), y3[:])

    nc.compile()
    return nc


def _prep_inputs(inputs):
    """Fold eval-mode BN into conv/linear weights; transpose for the device."""
    f = np.float32
    s = lambda g: (g / np.sqrt(f(1.0) + f(EPS))).astype(f)

    def edge(w, g, b, bias=None):
        O, C2 = w.shape
        C = C2 // 2
        sc = s(g)
        Wd = w[:, :C]
        Wc = w[:, C:]
        A = sc[:, None] * Wd
        Bm = sc[:, None] * (Wc - Wd)
        c0 = sc * (bias if bias is not None else 0.0) + b
        return A.T.copy().astype(f), Bm.T.copy().astype(f), c0.reshape(1, -1).astype(f)

    d = {}
    d["AT0"], d["BT0"], d["c00"] = edge(inputs["conv1_w"], inputs["bn1_g"],
                                        inputs["bn1_b"], inputs["conv1_b"])
    d["AT1"], d["BT1"], d["c01"] = edge(inputs["conv2_w"], inputs["bn2_g"], inputs["bn2_b"])
    d["AT2"], d["BT2"], d["c02"] = edge(inputs["conv3_w"], inputs["bn3_g"], inputs["bn3_b"])
    d["AT3"], d["BT3"], d["c03"] = edge(inputs["conv4_w"], inputs["bn4_g"], inputs["bn4_b"])
    h = np.float16
    d["w5T"] = inputs["conv5_w"].T.copy().astype(h)
    s6 = s(inputs["bn6_g"])
    d["l1T"] = (s6[:, None] * inputs["lin1_w"]).T.copy().astype(h)
    d["b6"] = inputs["bn6_b"].reshape(1, -1).astype(h)
    s7 = s(inputs["bn7_g"])
    d["l2T"] = (s7[:, None] * inputs["lin2_w"]).T.copy().astype(h)
    d["c7"] = (s7 * inputs["lin2_b"] + inputs["bn7_b"]).reshape(1, -1).astype(h)
    d["l3T"] = inputs["lin3_w"].T.copy().astype(h)
    d["b8"] = inputs["lin3_b"].reshape(1, -1).astype(h)
    return d


def _install_ntff_hook():
    """The agent image's antenv lacks axon_hooks; synthesize it and register
    the ctypes NTFF profiling hook from trn_agent_boot (same as trn_boot)."""
    import sys
    import types

    if "antenv.axon_hooks" in sys.modules:
        return
    import antenv

    mod = types.ModuleType("antenv.axon_hooks")
    holder = [None]
    mod.set_axon_ntff_profile_hook = lambda h: holder.__setitem__(0, h)
    mod.get_axon_ntff_profile_hook = lambda: holder[0]
    sys.modules["antenv.axon_hooks"] = mod
    antenv.axon_hooks = mod
    try:
        from trn_agent_boot.trn_boot import _ntff_profile_via_ctypes

        mod.set_axon_ntff_profile_hook(
            _ntff_profile_via_ctypes("/opt/axon/libaxon_pjrt.so"))
    except Exception as e:
        print(f"NTFF hook install failed: {e}")


def kernel(**inputs):
    global LAST_RESULTS
    from concourse.bass_utils import run_bass_kernel_spmd

    if "nc" not in _CACHE:
        _CACHE["nc"] = _build()
    nc = _CACHE["nc"]

    x = np.asarray(inputs["x"], dtype=np.float32)  # (8, 1024, 3)
    common = _prep_inputs({k: np.asarray(v) for k, v in inputs.items()})
    in_maps = [dict(common, xin=np.ascontiguousarray(x[i].T)) for i in range(NCORES)]

    trace = bool(int(os.environ.get("DGCNN_TRACE", "0")))
    if trace:
        _install_ntff_hook()
    res = run_bass_kernel_spmd(nc, in_maps, core_ids=list(range(NCORES)),
                               trace=trace, trace_cores=[0] if trace else None)
    LAST_RESULTS = res
    out = np.stack([r["out"].reshape(40) for r in res.results]).astype(np.float32)
    return out

